# revision 38
# baseline (speedup 1.0000x reference)
"""Multi-head attention (B=2, S=2048, d_model=768, H=12) on 8 TRN2 NeuronCores.

Sharding: 2-way data parallel over batch x 4-way tensor parallel over heads
(3 heads / 192-wide d_model slice per core). Host compacts masked keys away
(gather of unmasked key/value rows), pads to a 128 multiple, zero-fills pad
keys; softmax needs no mask handling on device (pad keys get V=0 and 0s in
the denominator ones-block). Host pre-arranges every input into the exact
SBUF layout (chunk-major, partition-contiguous) so each tensor loads as one
or two DMAs of 128 large packets; the output is likewise stored as one wide
DMA per query chunk and re-assembled on host.

Per core, a software-pipelined flat loop over (chunk c of 512 q, key tile t)
with emission order: exp01/exp2 (ACT) -> PV(i-lag) + norm (PE/DVE) ->
threaded projections (PE) -> scores(i+1) LAST. The PE queue therefore holds
pv+projection fill work ahead of scores(i+1), which is the only instruction
that has to wait for exp(i) (WAR on the scores PSUM tiles). sp is split into
sp01 (2 banks, heads 0/1, row-group-paired matmuls) and sp2 (1 bank, head 2)
so scores01(i+1) only waits on exp01(i). Steady-state period ~= the ACT busy
time per iteration.

V blocks of 128 cols/head, all [valid-ones 64 | V 64]: PV lands a
64-row-replicated denominator on partitions 0:64 and ctx on 64:128 of one
3-bank ctx PSUM tile (denominator costs no PE time - matmul cost scales
with N only). Normalization per head section: one wide fast reciprocal
straight from PSUM (partition base 0 only) -> psum*recip multiplies to bf16.
The first PV of each chunk trails two extra iterations so the previous
chunk's norm clears the ctx banks before the in-order PE reaches the
start=True PV.

Output projection runs transposed (outT[dm,q] = Wo_g^T @ ctx), in
adjacent-tile pairs so the two 64-contraction WO2 matmuls run in disjoint
PE row groups into different PSUM banks; bf16 results collect in one wide
SBUF tile per chunk and fly out as a single DMA. A burst of warm-up matmuls
on scratch data runs during the initial DMA wait so the PE HAM clock-gate
releases (1.2 -> 2.4 GHz) before the first real matmul arrives.
"""

import math

import numpy as np

B = 2
S = 2048
DM = 768
H = 12
DH = 64
G = 4              # head-group (tensor-parallel) degree
HPG = H // G       # heads per core
DQ = HPG * DH      # 192 d_model slice per core
NCORES = 8
P = 128
QC = 512           # query chunk
NQC = S // QC
NKT = DM // P      # 6 contraction tiles for projections
NMO = DM // P      # 6 output-projection tiles

_prog_cache = {}


def _build_nc(KP):
    import concourse.mybir as mybir
    import concourse.tile as tile
    from concourse import bacc

    F32 = mybir.dt.float32
    BF = mybir.dt.bfloat16
    AFT = mybir.ActivationFunctionType

    T = KP // P            # key tiles
    KCH = [(o, min(512, KP - o)) for o in range(0, KP, 512)]
    MW = 4 + T + DQ        # misc tensor cols: biases | vm | bv

    nc = bacc.Bacc(None, target_bir_lowering=False)
    # all inputs host-pre-arranged to [128, *] partition-contiguous layouts
    xq = nc.declare_dram_parameter("xq", [P, NQC * NKT * QC], BF, isOutput=False)
    xk = nc.declare_dram_parameter("xk", [P, NKT * KP], BF, isOutput=False)
    xv = nc.declare_dram_parameter("xv", [P, NKT * KP], BF, isOutput=False)
    wq = nc.declare_dram_parameter("wq", [P, NKT * DQ], BF, isOutput=False)
    wk = nc.declare_dram_parameter("wk", [P, NKT * DQ], BF, isOutput=False)
    wv = nc.declare_dram_parameter("wv", [P, NKT * DQ], BF, isOutput=False)
    wop = nc.declare_dram_parameter("wop", [P, 2 * DM], BF, isOutput=False)
    msc = nc.declare_dram_parameter("msc", [P, MW], F32, isOutput=False)
    out = nc.declare_dram_parameter("out", [P, NQC * NMO * QC], BF, isOutput=True)

    with tile.TileContext(nc) as tc:
        with (
            tc.tile_pool(name="persist", bufs=1) as persist,
            tc.tile_pool(name="es", bufs=6) as espool,
            tc.tile_pool(name="rc", bufs=4) as rcpool,
            tc.tile_pool(name="osb", bufs=2) as osb,
            tc.tile_pool(name="ps_sp", bufs=1, space="PSUM") as ps_sp,
            tc.tile_pool(name="ps_ctx", bufs=1, space="PSUM") as ps_ctx,
            tc.tile_pool(name="ps_w", bufs=2, space="PSUM") as ps_w,
        ):
            # ---- warm-up scratch + exp-table preload ----
            WUP = persist.tile([P, 512], BF, tag="WUP")
            nc.gpsimd.memset(WUP, 0.0)
            WRM = persist.tile([1, 2], F32, tag="WRM")
            nc.vector.memset(WRM, 0.0)
            nc.scalar.activation(WRM[:, 1:2], WRM[:, 0:1], AFT.Exp)
            for w in range(10):
                wps = ps_w.tile([P, 512], F32, tag="psw", name=f"warm{w}")
                nc.tensor.matmul(
                    wps, lhsT=WUP[:, 0:P], rhs=WUP, start=True, stop=True
                )

            # ---- constants / weights / activations (K path first) ----
            # DMA issue order tracks compute order: everything scores(0,0)
            # and exp(0,0) need first, then per-chunk pieces just in time.
            WK = persist.tile([P, NKT, DQ], BF, tag="WK")
            nc.sync.dma_start(out=WK, in_=wk[:, :].rearrange("p (kt m) -> p kt m", m=DQ))
            XKA = persist.tile([P, NKT * KP], BF, tag="XKA")
            c00, cw0 = KCH[0]
            h0w = (NKT // 2) * cw0
            nc.sync.dma_start(out=XKA[:, 0:h0w], in_=xk[:, 0:h0w])
            nc.sync.dma_start(out=XKA[:, h0w:NKT * cw0], in_=xk[:, h0w:NKT * cw0])
            MISC = persist.tile([P, MW], F32, tag="MISC")
            nc.sync.dma_start(out=MISC, in_=msc[:, :])
            WQ = persist.tile([P, NKT, DQ], BF, tag="WQ")
            nc.sync.dma_start(out=WQ, in_=wq[:, :].rearrange("p (kt m) -> p kt m", m=DQ))
            XQA = persist.tile([P, NQC * NKT * QC], BF, tag="XQA")
            nc.sync.dma_start(
                out=XQA[:, 0:NKT * QC], in_=xq[:, 0:NKT * QC]
            )
            if KP > cw0:
                nc.sync.dma_start(
                    out=XKA[:, NKT * cw0:], in_=xk[:, NKT * cw0:]
                )
            WV = persist.tile([P, NKT, DQ], BF, tag="WV")
            nc.sync.dma_start(out=WV, in_=wv[:, :].rearrange("p (kt m) -> p kt m", m=DQ))
            XVA = persist.tile([P, NKT * KP], BF, tag="XVA")
            nc.sync.dma_start(out=XVA[:, 0:NKT * cw0], in_=xv[:, 0:NKT * cw0])
            nc.sync.dma_start(
                out=XQA[:, NKT * QC:], in_=xq[:, NKT * QC:]
            )
            if KP > cw0:
                nc.sync.dma_start(
                    out=XVA[:, NKT * cw0:], in_=xv[:, NKT * cw0:]
                )
            WOP = persist.tile([P, 2 * DM], BF, tag="WOP")
            nc.sync.dma_start(out=WOP, in_=wop[:, :])

            # misc views
            BQ0 = MISC[:, 0:1]
            BQ1 = MISC[0:DH, 1:2]
            BK0 = MISC[:, 2:3]
            BK1 = MISC[0:DH, 3:4]
            VM = MISC[:, 4:4 + T]
            BV = MISC[:, 4 + T:4 + T + DQ]
            WO0 = WOP[:, 0:DM]          # wo rows 0:128 (h0,h1)
            WO2 = WOP[0:DH, DM:2 * DM]  # wo rows 128:192 (h2)

            # ---- persistent activations ----
            QT0 = persist.tile([P, S], BF, tag="QT0")    # heads 0,1
            QT1 = persist.tile([DH, S], BF, tag="QT1")   # head 2
            KT0 = persist.tile([P, KP], BF, tag="KT0")
            KT1 = persist.tile([DH, KP], BF, tag="KT1")  # head 2
            # V blocks of 128 cols per head (see module docstring)
            VP = persist.tile([P, T, HPG * P], BF, tag="VP")
            CTX01 = persist.tile([P, S], BF, tag="CTX01")  # h0 rows 0:64, h1 64:128
            CTX2 = persist.tile([DH, S], BF, tag="CTX2")   # h2

            ONES = persist.tile([P, HPG * DH], BF, tag="ONES")
            nc.vector.memset(ONES, 1.0)

            # ---- K projection (per key chunk; later chunks threaded into
            # the early attention iterations, DVE eviction) ----
            def kproj_m(kc, m, evict_on_act):
                c0, cw = KCH[kc]
                bias, mw = (BK0, P) if m == 0 else (BK1, DH)
                ps = ps_w.tile([P, 512], F32, tag="psw", name=f"kps{c0}_{m}")
                for kt in range(NKT):
                    nc.tensor.matmul(
                        ps[0:mw, 0:cw],
                        lhsT=WK[:, kt, m * P:m * P + mw],
                        rhs=XKA[:, c0 * NKT + kt * cw:c0 * NKT + (kt + 1) * cw],
                        start=(kt == 0),
                        stop=(kt == NKT - 1),
                    )
                dst = KT0 if m == 0 else KT1
                if evict_on_act:
                    nc.scalar.activation(
                        dst[0:mw, c0:c0 + cw], ps[0:mw, 0:cw],
                        AFT.Identity, bias=bias,
                    )
                else:
                    nc.vector.tensor_scalar_add(
                        dst[0:mw, c0:c0 + cw], ps[0:mw, 0:cw], bias
                    )

            def kproj_chunk(kc, evict_on_act):
                kproj_m(kc, 0, evict_on_act)
                kproj_m(kc, 1, evict_on_act)

            kproj_m(0, 0, True)

            _qps = {}

            def qproj_part(c, m, half, evict_on_act=False):
                c0 = c * QC
                bias, mw = (BQ0, P) if m == 0 else (BQ1, DH)
                if half == 0:
                    _qps[(c, m)] = ps_w.tile(
                        [P, 512], F32, tag="psw", name=f"qps{c}_{m}"
                    )
                ps = _qps[(c, m)]
                kts = range(0, NKT // 2) if half == 0 else range(NKT // 2, NKT)
                for kt in kts:
                    nc.tensor.matmul(
                        ps[0:mw, :],
                        lhsT=WQ[:, kt, m * P:m * P + mw],
                        rhs=XQA[:, (c * NKT + kt) * QC:(c * NKT + kt + 1) * QC],
                        start=(kt == 0),
                        stop=(kt == NKT - 1),
                    )
                if half == 0:
                    return
                dst = QT0 if m == 0 else QT1
                if evict_on_act:
                    nc.scalar.activation(
                        dst[0:mw, c0:c0 + QC], ps[0:mw, :], AFT.Identity, bias=bias
                    )
                else:
                    nc.vector.tensor_scalar_add(
                        dst[0:mw, c0:c0 + QC], ps[0:mw, :], bias
                    )
                del _qps[(c, m)]

            def qproj_half(c, m, evict_on_act=False):
                qproj_part(c, m, 0, evict_on_act)
                qproj_part(c, m, 1, evict_on_act)

            def vproj(t):
                kc = min(t * P // 512, len(KCH) - 1)
                c0, cw = KCH[kc]
                ps = ps_w.tile([P, 512], F32, tag="psw", name=f"vps{t}")
                for kt in range(NKT):
                    off = c0 * NKT + kt * cw + (t * P - c0)
                    nc.tensor.matmul(
                        ps[:, 0:DQ],
                        lhsT=XVA[:, off:off + P],
                        rhs=WV[:, kt, :],
                        start=(kt == 0),
                        stop=(kt == NKT - 1),
                    )
                # all blocks [valid-ones 0:64 | V 64:128]: PV rows 0:64 = den
                # (partition base 0 for the fused psum reciprocal), 64:128 ctx
                vview = VP[:, t, :].rearrange("p (h c) -> p h c", c=P)
                nc.vector.tensor_add(
                    vview[:, :, DH:P],
                    ps[:, 0:DQ].rearrange("p (h d) -> p h d", d=DH),
                    BV[:, :].rearrange("p (h d) -> p h d", d=DH),
                )
                nc.vector.tensor_scalar_mul(
                    vview[:, :, DH:P], vview[:, :, DH:P], VM[:, t:t + 1]
                )
                nc.vector.tensor_scalar_mul(
                    vview[:, :, 0:DH],
                    ONES[:, :].rearrange("p (h d) -> p h d", d=DH),
                    VM[:, t:t + 1],
                )

            # ---- attention, software-pipelined emission ----
            # sp split into two PSUM tiles so the WAR of scores(i+1) on
            # exp(i) releases per-section: scores01 only waits exp01.
            sp01 = ps_sp.tile([P, 1024], F32, tag="sp01")  # [h0 512 | h1 512]
            sp2 = ps_sp.tile([P, 512], F32, tag="sp2")     # h2
            ctx = ps_ctx.tile([P, 1536], F32, tag="ctx")   # [h0 | h1 | h2]
            SCL = 1.0 / math.sqrt(DH)

            def scores01(c, t):
                c0 = c * QC
                tsl = slice(t * P, (t + 1) * P)
                nc.tensor.matmul(
                    sp01[:, 0:512],
                    lhsT=KT0[0:DH, tsl], rhs=QT0[0:DH, c0:c0 + QC],
                    start=True, stop=True,
                )
                nc.tensor.matmul(
                    sp01[:, 512:1024],
                    lhsT=KT0[DH:P, tsl], rhs=QT0[DH:P, c0:c0 + QC],
                    start=True, stop=True,
                )

            def scoresh2(c, t):
                c0 = c * QC
                tsl = slice(t * P, (t + 1) * P)
                nc.tensor.matmul(
                    sp2[:, 0:512],
                    lhsT=KT1[0:DH, tsl], rhs=QT1[0:DH, c0:c0 + QC],
                    start=True, stop=True,
                )

            _osb = {}

            def osb_for(c):
                if c not in _osb:
                    _osb[c] = osb.tile(
                        [P, NMO * QC], BF, tag="posb", name=f"osb{c}"
                    )
                return _osb[c]

            def oproj_flush(c):
                nc.sync.dma_start(
                    out=out[:, c * NMO * QC:(c + 1) * NMO * QC],
                    in_=_osb.pop(c),
                )

            def oproj_one(c, mo, evict_act=False):
                # outT[mo*128:(mo+1)*128, chunk c] = WO0[:,mo].T@CTX01 + WO2[:,mo].T@CTX2
                c0 = c * QC
                po = ps_w.tile([P, 512], F32, tag="psw", name=f"po{c}_{mo}")
                osl = slice(mo * DM, mo * DM + DM)
                nc.tensor.matmul(
                    po, lhsT=WO0[:, mo * P:(mo + 1) * P],
                    rhs=CTX01[:, c0:c0 + QC],
                    start=True, stop=False,
                )
                nc.tensor.matmul(
                    po, lhsT=WO2[:, mo * P:(mo + 1) * P],
                    rhs=CTX2[0:DH, c0:c0 + QC],
                    start=False, stop=True,
                )
                dst = osb_for(c)[:, mo * QC:(mo + 1) * QC]
                if evict_act:
                    nc.scalar.activation(dst, po, AFT.Identity, bias=0.0)
                else:
                    nc.vector.tensor_copy(dst, po)

            def pv_one(c, t, h):
                e01, e2 = es_hist[c * T + t]
                rhs = e01[:, h * 512:(h + 1) * 512] if h < 2 else e2[:, 0:512]
                nc.tensor.matmul(
                    ctx[:, h * 512:h * 512 + 512],
                    lhsT=VP[:, t, h * P:(h + 1) * P],
                    rhs=rhs,
                    start=(t == 0), stop=(t == T - 1),
                )

            _NRM = [
                (CTX01, slice(0, DH)),
                (CTX01, slice(DH, P)),
                (CTX2, slice(0, DH)),
            ]

            def norm_h(c, h):
                # den rows 0:64 of this head's ctx section (partition base
                # 0): fused reciprocal psum->sbuf, then one multiply.
                c0 = c * QC
                cs = slice(h * 512, (h + 1) * 512)
                rc = rcpool.tile([DH, 512], F32, tag="rc", name=f"rc{c}_{h}")
                nc.vector.reciprocal_approx_fast(rc, ctx[0:DH, cs])
                dst, dr = _NRM[h]
                nc.vector.tensor_mul(
                    dst[dr, c0:c0 + QC], ctx[DH:P, cs], rc
                )

            # Flat pipeline over i = c*T + t. Emission order per iteration:
            # exp01(i), exp2(i) [ACT] -> PV + per-head norm [PE/DVE] ->
            # threaded projections [PE] -> scores(i+1) -> chunk-opening PVs
            # LAST. Head h's PV trails by 1+h iterations (head skew): each
            # head finishes its chunk - and its norm slice - on a different
            # iteration, so normalization and the next chunk's start=True
            # PVs overlap the boundary instead of serializing behind one big
            # norm. t==0 PVs (which WAR the previous chunk's norm) go after
            # scores so they never head-of-line block the PE queue.
            NI = NQC * T
            es_hist = {}
            # interleave the startup so scores(0,0) - and with it the exp
            # pipeline - starts as soon as the m0 halves of K/Q are ready;
            # kproj m1 runs before qproj so the PE never idles into a HAM
            # re-throttle while the first q chunk is still in flight
            kproj_m(0, 1, True)
            qproj_half(0, 0, evict_on_act=True)
            scores01(0, 0)
            qproj_half(0, 1, evict_on_act=True)
            scoresh2(0, 0)
            for i in range(NI + 1 + HPG - 1):
                c, t = divmod(min(i, NI - 1), T)
                if i < NI:
                    e01 = espool.tile(
                        [P, 1024], BF, tag="es01", name=f"es01_{c}_{t}"
                    )
                    e2 = espool.tile(
                        [P, 512], BF, tag="es2", name=f"es2_{c}_{t}"
                    )
                    es_hist[c * T + t] = (e01, e2)
                    nc.scalar.activation(
                        e01, sp01, AFT.Exp, bias=0.0, scale=SCL
                    )
                    nc.scalar.activation(
                        e2, sp2, AFT.Exp, bias=0.0, scale=SCL
                    )
                late_pv = []
                norms_due = []
                for h in range(HPG):
                    jh = i - 1 - h
                    if not (0 <= jh < NI):
                        continue
                    ch, th = divmod(jh, T)
                    if th == 0 and ch > 0:
                        late_pv.append((ch, th, h, jh))
                        continue
                    pv_one(ch, th, h)
                    if h == HPG - 1:
                        es_hist.pop(jh)
                    if th == T - 1:
                        norms_due.append((ch, h))
                for ch, h in norms_due:
                    norm_h(ch, h)
                # threaded non-attention work, keyed by the exp index (c, t)
                if c == 0 and i < NI and t + 1 < len(KCH):
                    kproj_chunk(t + 1, False)
                if c == 0 and i < NI:
                    if t == 0:
                        vproj(0)
                        if T > 1:
                            vproj(1)
                    elif t + 1 < T:
                        vproj(t + 1)
                if c > 0 and i < NI and 2 <= t <= min(T - 1, NMO + 1):
                    oproj_one(c - 1, t - 2)
                    if t - 2 == NMO - 1:
                        oproj_flush(c - 1)
                if c + 1 < NQC and T >= 3 and i < NI:
                    if t == T - 3:
                        qproj_half(c + 1, 0)
                    elif t == T - 2:
                        qproj_half(c + 1, 1)
                if c + 1 < NQC and T < 3 and i < NI and t == T - 1:
                    qproj_half(c + 1, 0)
                    qproj_half(c + 1, 1)
                # leftover O-proj tiles when T is too small to thread them all
                if c > 0 and t == T - 1 and i < NI and T - 2 < NMO:
                    for mo in range(max(T - 2, 0), NMO):
                        oproj_one(c - 1, mo)
                    oproj_flush(c - 1)
                # next scores, then the chunk-opening PVs dead last (they
                # are gated on the previous chunk's norm)
                if i + 1 < NI:
                    scores01(*divmod(i + 1, T))
                    scoresh2(*divmod(i + 1, T))
                for ch, th, h, jh in late_pv:
                    pv_one(ch, th, h)
                    if h == HPG - 1:
                        es_hist.pop(jh)
                    if th == T - 1:
                        norm_h(ch, h)
            # tail: keep the PE HAM-warm with scratch matmuls while the
            # last norms drain on DVE, then the last chunk's O-projection
            # in pairs - both WO0 matmuls (only need CTX01) ahead of the
            # WO2 matmuls (need norm_h2), evictions alternating ACT/DVE,
            # out-DMA per pair so the store overlaps the remaining compute
            for w in range(10):
                wps = ps_w.tile([P, 512], F32, tag="psw", name=f"tw{w}")
                nc.tensor.matmul(
                    wps, lhsT=WUP[:, 0:P], rhs=WUP, start=True, stop=True
                )
            lc = NQC - 1
            c0l = lc * QC
            for mo in range(0, NMO, 2):
                pos = []
                for k in range(2):
                    po = ps_w.tile(
                        [P, 512], F32, tag="psw", name=f"tl{mo + k}"
                    )
                    nc.tensor.matmul(
                        po, lhsT=WO0[:, (mo + k) * P:(mo + k + 1) * P],
                        rhs=CTX01[:, c0l:c0l + QC],
                        start=True, stop=False,
                    )
                    pos.append(po)
                for k in range(2):
                    nc.tensor.matmul(
                        pos[k], lhsT=WO2[:, (mo + k) * P:(mo + k + 1) * P],
                        rhs=CTX2[0:DH, c0l:c0l + QC],
                        start=False, stop=True,
                    )
                ot = osb_for(lc)
                for k in range(2):
                    dst = ot[:, (mo + k) * QC:(mo + k + 1) * QC]
                    if k == 0:
                        nc.scalar.activation(dst, pos[k], AFT.Identity, bias=0.0)
                    else:
                        nc.vector.tensor_copy(dst, pos[k])
                nc.sync.dma_start(
                    out=out[:, (lc * NMO + mo) * QC:(lc * NMO + mo + 2) * QC],
                    in_=ot[:, mo * QC:(mo + 2) * QC],
                )
            _osb.pop(lc)
    nc.compile()
    return nc


def _get_prog(KP):
    if KP not in _prog_cache:
        _prog_cache[KP] = _build_nc(KP)
    return _prog_cache[KP]


def _rearrange_w(Wslice, BF):
    # [768, 192] -> [128, 6*192] (p-major kt blocks), contiguous for DMA
    return np.ascontiguousarray(
        Wslice.reshape(DM // P, P, DQ).transpose(1, 0, 2).reshape(P, -1)
    ).astype(BF)


def _chunk_major(xT, KCH):
    # [768, KP] -> [128, NKT*KP] with per-partition layout [kc][kt][cols]
    x3 = xT.reshape(NKT, P, xT.shape[1])
    return np.concatenate(
        [
            np.ascontiguousarray(
                x3[:, :, c0:c0 + cw].transpose(1, 0, 2)
            ).reshape(P, NKT * cw)
            for c0, cw in KCH
        ],
        axis=1,
    )


def _run(inputs, trace=False):
    import ml_dtypes
    from concourse.bass_utils import run_bass_kernel_spmd

    BF = ml_dtypes.bfloat16

    query = np.asarray(inputs["query"], dtype=np.float32)
    key = np.asarray(inputs["key"], dtype=np.float32)
    value = np.asarray(inputs["value"], dtype=np.float32)
    mask = np.asarray(inputs["mask"])
    Wq = np.asarray(inputs["Wq"], dtype=np.float32)
    bq = np.asarray(inputs["bq"], dtype=np.float32)
    Wk = np.asarray(inputs["Wk"], dtype=np.float32)
    bk = np.asarray(inputs["bk"], dtype=np.float32)
    Wv = np.asarray(inputs["Wv"], dtype=np.float32)
    bv = np.asarray(inputs["bv"], dtype=np.float32)
    Wo = np.asarray(inputs["Wo"], dtype=np.float32)
    bo = np.asarray(inputs["bo"], dtype=np.float32)

    idx = [np.nonzero(mask[b, 0, 0] != 0)[0] for b in range(B)]
    keff = [len(i) for i in idx]
    KP = max(P, ((max(keff) + P - 1) // P) * P)
    T = KP // P
    KCH = [(o, min(512, KP - o)) for o in range(0, KP, 512)]

    nc = _get_prog(KP)

    per_batch = {}
    for b in range(B):
        # q: [128, NQC*NKT*QC] chunk-major ([c][kt][q])
        xqT = query[b].T.astype(BF)  # [768, 2048]
        xq_p = np.ascontiguousarray(
            xqT.reshape(NKT, P, NQC, QC).transpose(1, 2, 0, 3)
        ).reshape(P, -1)
        xkT = np.zeros((DM, KP), dtype=BF)
        xkT[:, :keff[b]] = key[b][idx[b]].T.astype(BF)
        xvT = np.zeros((DM, KP), dtype=BF)
        xvT[:, :keff[b]] = value[b][idx[b]].T.astype(BF)
        xk_p = _chunk_major(xkT, KCH)
        xv_p = _chunk_major(xvT, KCH)
        vmf = np.zeros((KP,), dtype=np.float32)
        vmf[:keff[b]] = 1.0
        vm2 = np.ascontiguousarray(vmf.reshape(T, P).T)  # [128, T]
        per_batch[b] = (xq_p, xk_p, xv_p, vm2)

    in_maps = []
    for core in range(NCORES):
        b, g = core // G, core % G
        xq_p, xk_p, xv_p, vm2 = per_batch[b]
        sl = slice(g * DQ, (g + 1) * DQ)
        wo_g = Wo[sl, :].astype(BF)  # [192, 768]
        wo_pack = np.concatenate(
            [wo_g[0:P], np.concatenate([wo_g[P:DQ], wo_g[P:DQ]], axis=0)],
            axis=1,
        )  # [128, 1536]
        mw = 4 + T + DQ
        msc = np.zeros((P, mw), dtype=np.float32)
        msc[:, 0] = bq[sl][0:P]
        msc[0:DH, 1] = bq[sl][P:DQ]
        msc[:, 2] = bk[sl][0:P]
        msc[0:DH, 3] = bk[sl][P:DQ]
        msc[:, 4:4 + T] = vm2
        msc[:, 4 + T:] = bv[sl][None, :]
        in_maps.append({
            "xq": xq_p,
            "xk": xk_p,
            "xv": xv_p,
            "wq": _rearrange_w(Wq[:, sl], BF),
            "wk": _rearrange_w(Wk[:, sl], BF),
            "wv": _rearrange_w(Wv[:, sl], BF),
            "wop": np.ascontiguousarray(wo_pack),
            "msc": msc,
        })

    res = run_bass_kernel_spmd(nc, in_maps, list(range(NCORES)), trace=trace)

    outp = np.zeros((B, S, DM), dtype=np.float32)
    for core in range(NCORES):
        o = np.asarray(res.results[core]["out"], dtype=np.float32)
        outT = o.reshape(P, NQC, NMO, QC).transpose(2, 0, 1, 3).reshape(DM, S)
        outp[core // G] += outT.T
    outp += bo.reshape(1, 1, DM)
    return outp, res


def kernel(**inputs) -> np.ndarray:
    out, _ = _run(inputs, trace=False)
    return out


if __name__ == "__main__":
    nc = _build_nc(1152)
    print("build OK")


# revision 41
# speedup vs baseline: 1.0290x; 1.0290x over previous
"""Multi-head attention (B=2, S=2048, d_model=768, H=12) on 8 TRN2 NeuronCores.

Sharding: 2-way data parallel over batch x 4-way tensor parallel over heads
(3 heads / 192-wide d_model slice per core). Host compacts masked keys away
(gather of unmasked key/value rows), pads to a 128 multiple, zero-fills pad
keys; softmax needs no mask handling on device (pad keys get V=0 and 0s in
the denominator ones-block). Host pre-arranges every input into the exact
SBUF layout (chunk-major, partition-contiguous) so each tensor loads as one
or two DMAs of 128 large packets; the output is likewise stored as one wide
DMA per query chunk and re-assembled on host.

Per core, a software-pipelined flat loop over (chunk c of 512 q, key tile t)
with emission order: exp01/exp2 (ACT) -> PV(i-lag) + norm (PE/DVE) ->
threaded projections (PE) -> scores(i+1) LAST. The PE queue therefore holds
pv+projection fill work ahead of scores(i+1), which is the only instruction
that has to wait for exp(i) (WAR on the scores PSUM tiles). sp is split into
sp01 (2 banks, heads 0/1, row-group-paired matmuls) and sp2 (1 bank, head 2)
so scores01(i+1) only waits on exp01(i). Steady-state period ~= the ACT busy
time per iteration.

V blocks of 128 cols/head, all [valid-ones 64 | V 64]: PV lands a
64-row-replicated denominator on partitions 0:64 and ctx on 64:128 of one
3-bank ctx PSUM tile (denominator costs no PE time - matmul cost scales
with N only). Normalization per head section: one wide fast reciprocal
straight from PSUM (partition base 0 only) -> psum*recip multiplies to bf16.
The first PV of each chunk trails two extra iterations so the previous
chunk's norm clears the ctx banks before the in-order PE reaches the
start=True PV.

Output projection runs transposed (outT[dm,q] = Wo_g^T @ ctx), in
adjacent-tile pairs so the two 64-contraction WO2 matmuls run in disjoint
PE row groups into different PSUM banks; bf16 results collect in one wide
SBUF tile per chunk and fly out as a single DMA. A burst of warm-up matmuls
on scratch data runs during the initial DMA wait so the PE HAM clock-gate
releases (1.2 -> 2.4 GHz) before the first real matmul arrives.
"""

import math

import numpy as np

B = 2
S = 2048
DM = 768
H = 12
DH = 64
G = 4              # head-group (tensor-parallel) degree
HPG = H // G       # heads per core
DQ = HPG * DH      # 192 d_model slice per core
NCORES = 8
P = 128
QC = 512           # query chunk
NQC = S // QC
NKT = DM // P      # 6 contraction tiles for projections
NMO = DM // P      # 6 output-projection tiles

_prog_cache = {}


def _build_nc(KP):
    import concourse.mybir as mybir
    import concourse.tile as tile
    from concourse import bacc

    F32 = mybir.dt.float32
    BF = mybir.dt.bfloat16
    AFT = mybir.ActivationFunctionType

    T = KP // P            # key tiles
    KCH = [(o, min(512, KP - o)) for o in range(0, KP, 512)]
    MW = 4 + T + DQ        # misc tensor cols: biases | vm | bv

    nc = bacc.Bacc(None, target_bir_lowering=False)
    # all inputs host-pre-arranged to [128, *] partition-contiguous layouts
    xq = nc.declare_dram_parameter("xq", [P, NQC * NKT * QC], BF, isOutput=False)
    xk = nc.declare_dram_parameter("xk", [P, NKT * KP], BF, isOutput=False)
    xv = nc.declare_dram_parameter("xv", [P, NKT * KP], BF, isOutput=False)
    wq = nc.declare_dram_parameter("wq", [P, NKT * DQ], BF, isOutput=False)
    wk = nc.declare_dram_parameter("wk", [P, NKT * DQ], BF, isOutput=False)
    wv = nc.declare_dram_parameter("wv", [P, NKT * DQ], BF, isOutput=False)
    wop = nc.declare_dram_parameter("wop", [P, 2 * DM], BF, isOutput=False)
    msc = nc.declare_dram_parameter("msc", [P, MW], F32, isOutput=False)
    out = nc.declare_dram_parameter("out", [P, NQC * NMO * QC], BF, isOutput=True)

    with tile.TileContext(nc) as tc:
        with (
            tc.tile_pool(name="persist", bufs=1) as persist,
            tc.tile_pool(name="es", bufs=6) as espool,
            tc.tile_pool(name="rc", bufs=4) as rcpool,
            tc.tile_pool(name="osb", bufs=2) as osb,
            tc.tile_pool(name="ps_sp", bufs=1, space="PSUM") as ps_sp,
            tc.tile_pool(name="ps_ctx", bufs=1, space="PSUM") as ps_ctx,
            tc.tile_pool(name="ps_w", bufs=2, space="PSUM") as ps_w,
        ):
            # ---- warm-up scratch + exp-table preload ----
            WUP = persist.tile([P, 512], BF, tag="WUP")
            nc.gpsimd.memset(WUP, 0.0)
            WRM = persist.tile([1, 2], F32, tag="WRM")
            nc.vector.memset(WRM, 0.0)
            nc.scalar.activation(WRM[:, 1:2], WRM[:, 0:1], AFT.Exp)
            for w in range(10):
                wps = ps_w.tile([P, 512], F32, tag="psw", name=f"warm{w}")
                nc.tensor.matmul(
                    wps, lhsT=WUP[:, 0:P], rhs=WUP, start=True, stop=True
                )

            # ---- constants / weights / activations (K path first) ----
            # DMA issue order tracks compute order: everything scores(0,0)
            # and exp(0,0) need first, then per-chunk pieces just in time.
            WK = persist.tile([P, NKT, DQ], BF, tag="WK")
            nc.sync.dma_start(out=WK, in_=wk[:, :].rearrange("p (kt m) -> p kt m", m=DQ))
            XKA = persist.tile([P, NKT * KP], BF, tag="XKA")
            c00, cw0 = KCH[0]
            h0w = (NKT // 2) * cw0
            nc.sync.dma_start(out=XKA[:, 0:h0w], in_=xk[:, 0:h0w])
            nc.sync.dma_start(out=XKA[:, h0w:NKT * cw0], in_=xk[:, h0w:NKT * cw0])
            MISC = persist.tile([P, MW], F32, tag="MISC")
            nc.sync.dma_start(out=MISC, in_=msc[:, :])
            WQ = persist.tile([P, NKT, DQ], BF, tag="WQ")
            nc.sync.dma_start(out=WQ, in_=wq[:, :].rearrange("p (kt m) -> p kt m", m=DQ))
            XQA = persist.tile([P, NQC * NKT * QC], BF, tag="XQA")
            nc.sync.dma_start(
                out=XQA[:, 0:NKT * QC], in_=xq[:, 0:NKT * QC]
            )
            if KP > cw0:
                nc.sync.dma_start(
                    out=XKA[:, NKT * cw0:], in_=xk[:, NKT * cw0:]
                )
            WV = persist.tile([P, NKT, DQ], BF, tag="WV")
            nc.sync.dma_start(out=WV, in_=wv[:, :].rearrange("p (kt m) -> p kt m", m=DQ))
            XVA = persist.tile([P, NKT * KP], BF, tag="XVA")
            nc.sync.dma_start(out=XVA[:, 0:NKT * cw0], in_=xv[:, 0:NKT * cw0])
            nc.sync.dma_start(
                out=XQA[:, NKT * QC:], in_=xq[:, NKT * QC:]
            )
            if KP > cw0:
                nc.sync.dma_start(
                    out=XVA[:, NKT * cw0:], in_=xv[:, NKT * cw0:]
                )
            WOP = persist.tile([P, 2 * DM], BF, tag="WOP")
            nc.sync.dma_start(out=WOP, in_=wop[:, :])

            # misc views
            BQ0 = MISC[:, 0:1]
            BQ1 = MISC[0:DH, 1:2]
            BK0 = MISC[:, 2:3]
            BK1 = MISC[0:DH, 3:4]
            VM = MISC[:, 4:4 + T]
            BV = MISC[:, 4 + T:4 + T + DQ]
            WO0 = WOP[:, 0:DM]          # wo rows 0:128 (h0,h1)
            WO2 = WOP[0:DH, DM:2 * DM]  # wo rows 128:192 (h2)

            # ---- persistent activations ----
            QT0 = persist.tile([P, S], BF, tag="QT0")    # heads 0,1
            QT1 = persist.tile([DH, S], BF, tag="QT1")   # head 2
            KT0 = persist.tile([P, KP], BF, tag="KT0")
            KT1 = persist.tile([DH, KP], BF, tag="KT1")  # head 2
            # V blocks of 128 cols per head (see module docstring)
            VP = persist.tile([P, T, HPG * P], BF, tag="VP")
            CTX01 = persist.tile([P, S], BF, tag="CTX01")  # h0 rows 0:64, h1 64:128
            CTX2 = persist.tile([DH, S], BF, tag="CTX2")   # h2

            ONES = persist.tile([P, HPG * DH], BF, tag="ONES")
            nc.vector.memset(ONES, 1.0)

            # ---- K projection (per key chunk; later chunks threaded into
            # the early attention iterations, DVE eviction) ----
            def kproj_m(kc, m, evict_on_act):
                c0, cw = KCH[kc]
                bias, mw = (BK0, P) if m == 0 else (BK1, DH)
                ps = ps_w.tile([P, 512], F32, tag="psw", name=f"kps{c0}_{m}")
                for kt in range(NKT):
                    nc.tensor.matmul(
                        ps[0:mw, 0:cw],
                        lhsT=WK[:, kt, m * P:m * P + mw],
                        rhs=XKA[:, c0 * NKT + kt * cw:c0 * NKT + (kt + 1) * cw],
                        start=(kt == 0),
                        stop=(kt == NKT - 1),
                    )
                dst = KT0 if m == 0 else KT1
                if evict_on_act:
                    nc.scalar.activation(
                        dst[0:mw, c0:c0 + cw], ps[0:mw, 0:cw],
                        AFT.Identity, bias=bias,
                    )
                else:
                    nc.vector.tensor_scalar_add(
                        dst[0:mw, c0:c0 + cw], ps[0:mw, 0:cw], bias
                    )

            def kproj_chunk(kc, evict_on_act):
                kproj_m(kc, 0, evict_on_act)
                kproj_m(kc, 1, evict_on_act)

            kproj_m(0, 0, True)

            _qps = {}

            def qproj_part(c, m, half, evict_on_act=False):
                c0 = c * QC
                bias, mw = (BQ0, P) if m == 0 else (BQ1, DH)
                if half == 0:
                    _qps[(c, m)] = ps_w.tile(
                        [P, 512], F32, tag="psw", name=f"qps{c}_{m}"
                    )
                ps = _qps[(c, m)]
                kts = range(0, NKT // 2) if half == 0 else range(NKT // 2, NKT)
                for kt in kts:
                    nc.tensor.matmul(
                        ps[0:mw, :],
                        lhsT=WQ[:, kt, m * P:m * P + mw],
                        rhs=XQA[:, (c * NKT + kt) * QC:(c * NKT + kt + 1) * QC],
                        start=(kt == 0),
                        stop=(kt == NKT - 1),
                    )
                if half == 0:
                    return
                dst = QT0 if m == 0 else QT1
                if evict_on_act:
                    nc.scalar.activation(
                        dst[0:mw, c0:c0 + QC], ps[0:mw, :], AFT.Identity, bias=bias
                    )
                else:
                    nc.vector.tensor_scalar_add(
                        dst[0:mw, c0:c0 + QC], ps[0:mw, :], bias
                    )
                del _qps[(c, m)]

            def qproj_half(c, m, evict_on_act=False):
                qproj_part(c, m, 0, evict_on_act)
                qproj_part(c, m, 1, evict_on_act)

            def vproj(t):
                kc = min(t * P // 512, len(KCH) - 1)
                c0, cw = KCH[kc]
                ps = ps_w.tile([P, 512], F32, tag="psw", name=f"vps{t}")
                for kt in range(NKT):
                    off = c0 * NKT + kt * cw + (t * P - c0)
                    nc.tensor.matmul(
                        ps[:, 0:DQ],
                        lhsT=XVA[:, off:off + P],
                        rhs=WV[:, kt, :],
                        start=(kt == 0),
                        stop=(kt == NKT - 1),
                    )
                # all blocks [valid-ones 0:64 | V 64:128]: PV rows 0:64 = den
                # (partition base 0 for the fused psum reciprocal), 64:128 ctx
                vview = VP[:, t, :].rearrange("p (h c) -> p h c", c=P)
                nc.vector.tensor_add(
                    vview[:, :, DH:P],
                    ps[:, 0:DQ].rearrange("p (h d) -> p h d", d=DH),
                    BV[:, :].rearrange("p (h d) -> p h d", d=DH),
                )
                nc.vector.tensor_scalar_mul(
                    vview[:, :, DH:P], vview[:, :, DH:P], VM[:, t:t + 1]
                )
                nc.vector.tensor_scalar_mul(
                    vview[:, :, 0:DH],
                    ONES[:, :].rearrange("p (h d) -> p h d", d=DH),
                    VM[:, t:t + 1],
                )

            # ---- attention, software-pipelined emission ----
            # sp split into two PSUM tiles so the WAR of scores(i+1) on
            # exp(i) releases per-section: scores01 only waits exp01.
            sp01 = ps_sp.tile([P, 1024], F32, tag="sp01")  # [h0 512 | h1 512]
            sp2 = ps_sp.tile([P, 512], F32, tag="sp2")     # h2
            ctx = ps_ctx.tile([P, 1536], F32, tag="ctx")   # [h0 | h1 | h2]
            SCL = 1.0 / math.sqrt(DH)

            def scores01(c, t):
                c0 = c * QC
                tsl = slice(t * P, (t + 1) * P)
                nc.tensor.matmul(
                    sp01[:, 0:512],
                    lhsT=KT0[0:DH, tsl], rhs=QT0[0:DH, c0:c0 + QC],
                    start=True, stop=True,
                )
                nc.tensor.matmul(
                    sp01[:, 512:1024],
                    lhsT=KT0[DH:P, tsl], rhs=QT0[DH:P, c0:c0 + QC],
                    start=True, stop=True,
                )

            def scoresh2(c, t):
                c0 = c * QC
                tsl = slice(t * P, (t + 1) * P)
                nc.tensor.matmul(
                    sp2[:, 0:512],
                    lhsT=KT1[0:DH, tsl], rhs=QT1[0:DH, c0:c0 + QC],
                    start=True, stop=True,
                )

            _osb = {}

            def osb_for(c):
                if c not in _osb:
                    _osb[c] = osb.tile(
                        [P, NMO * QC], BF, tag="posb", name=f"osb{c}"
                    )
                return _osb[c]

            def oproj_flush(c):
                nc.sync.dma_start(
                    out=out[:, c * NMO * QC:(c + 1) * NMO * QC],
                    in_=_osb.pop(c),
                )

            def oproj_one(c, mo, evict_act=False):
                # outT[mo*128:(mo+1)*128, chunk c] = WO0[:,mo].T@CTX01 + WO2[:,mo].T@CTX2
                c0 = c * QC
                po = ps_w.tile([P, 512], F32, tag="psw", name=f"po{c}_{mo}")
                osl = slice(mo * DM, mo * DM + DM)
                nc.tensor.matmul(
                    po, lhsT=WO0[:, mo * P:(mo + 1) * P],
                    rhs=CTX01[:, c0:c0 + QC],
                    start=True, stop=False,
                )
                nc.tensor.matmul(
                    po, lhsT=WO2[:, mo * P:(mo + 1) * P],
                    rhs=CTX2[0:DH, c0:c0 + QC],
                    start=False, stop=True,
                )
                dst = osb_for(c)[:, mo * QC:(mo + 1) * QC]
                if evict_act:
                    nc.scalar.activation(dst, po, AFT.Identity, bias=0.0)
                else:
                    nc.vector.tensor_copy(dst, po)

            def pv_one(c, t, h):
                e01, e2 = es_hist[c * T + t]
                rhs = e01[:, h * 512:(h + 1) * 512] if h < 2 else e2[:, 0:512]
                nc.tensor.matmul(
                    ctx[:, h * 512:h * 512 + 512],
                    lhsT=VP[:, t, h * P:(h + 1) * P],
                    rhs=rhs,
                    start=(t == 0), stop=(t == T - 1),
                )

            _NRM = [
                (CTX01, slice(0, DH)),
                (CTX01, slice(DH, P)),
                (CTX2, slice(0, DH)),
            ]

            def norm_h(c, h):
                # den rows 0:64 of this head's ctx section (partition base
                # 0): fused reciprocal psum->sbuf, then one multiply.
                c0 = c * QC
                cs = slice(h * 512, (h + 1) * 512)
                rc = rcpool.tile([DH, 512], F32, tag="rc", name=f"rc{c}_{h}")
                nc.vector.reciprocal_approx_fast(rc, ctx[0:DH, cs])
                dst, dr = _NRM[h]
                nc.vector.tensor_mul(
                    dst[dr, c0:c0 + QC], ctx[DH:P, cs], rc
                )

            # Flat pipeline over i = c*T + t. Emission order per iteration:
            # exp01(i), exp2(i) [ACT] -> PV + per-head norm [PE/DVE] ->
            # threaded projections [PE] -> scores(i+1) -> chunk-opening PVs
            # LAST. Head h's PV trails by 1+h iterations (head skew): each
            # head finishes its chunk - and its norm slice - on a different
            # iteration, so normalization and the next chunk's start=True
            # PVs overlap the boundary instead of serializing behind one big
            # norm. t==0 PVs (which WAR the previous chunk's norm) go after
            # scores so they never head-of-line block the PE queue.
            NI = NQC * T
            es_hist = {}
            # interleave the startup so scores(0,0) - and with it the exp
            # pipeline - starts as soon as the m0 halves of K/Q are ready;
            # kproj m1 runs before qproj, and a second burst of scratch
            # matmuls bridges the wait for the first q chunk so the PE
            # never idles into a HAM re-throttle
            kproj_m(0, 1, True)
            for w in range(8):
                wps = ps_w.tile([P, 512], F32, tag="psw", name=f"warmb{w}")
                nc.tensor.matmul(
                    wps, lhsT=WUP[:, 0:P], rhs=WUP, start=True, stop=True
                )
            qproj_half(0, 0, evict_on_act=True)
            scores01(0, 0)
            qproj_half(0, 1, evict_on_act=True)
            scoresh2(0, 0)
            for i in range(NI + 1 + HPG - 1):
                c, t = divmod(min(i, NI - 1), T)
                if i < NI:
                    e01 = espool.tile(
                        [P, 1024], BF, tag="es01", name=f"es01_{c}_{t}"
                    )
                    e2 = espool.tile(
                        [P, 512], BF, tag="es2", name=f"es2_{c}_{t}"
                    )
                    es_hist[c * T + t] = (e01, e2)
                    nc.scalar.activation(
                        e01, sp01, AFT.Exp, bias=0.0, scale=SCL
                    )
                    nc.scalar.activation(
                        e2, sp2, AFT.Exp, bias=0.0, scale=SCL
                    )
                if i >= NI:
                    # drain iterations: scratch matmuls keep the HAM clock
                    # released while the final norms run on DVE
                    for w in range(4):
                        wps = ps_w.tile(
                            [P, 512], F32, tag="psw", name=f"dw{i}_{w}"
                        )
                        nc.tensor.matmul(
                            wps, lhsT=WUP[:, 0:P], rhs=WUP,
                            start=True, stop=True,
                        )
                late_pv = []
                norms_due = []
                for h in range(HPG):
                    jh = i - 1 - h
                    if not (0 <= jh < NI):
                        continue
                    ch, th = divmod(jh, T)
                    if th == 0 and ch > 0:
                        late_pv.append((ch, th, h, jh))
                        continue
                    pv_one(ch, th, h)
                    if h == HPG - 1:
                        es_hist.pop(jh)
                    if th == T - 1:
                        norms_due.append((ch, h))
                for ch, h in norms_due:
                    norm_h(ch, h)
                # threaded non-attention work, keyed by the exp index (c, t)
                if c == 0 and i < NI and t + 1 < len(KCH):
                    kproj_chunk(t + 1, False)
                if c == 0 and i < NI:
                    if t == 0:
                        vproj(0)
                        if T > 1:
                            vproj(1)
                    elif t + 1 < T:
                        vproj(t + 1)
                if c > 0 and i < NI and 2 <= t <= min(T - 1, NMO + 1):
                    oproj_one(c - 1, t - 2)
                    if t - 2 == NMO - 1:
                        oproj_flush(c - 1)
                if c + 1 < NQC and T >= 3 and i < NI:
                    if t == T - 3:
                        qproj_half(c + 1, 0)
                    elif t == T - 2:
                        qproj_half(c + 1, 1)
                if c + 1 < NQC and T < 3 and i < NI and t == T - 1:
                    qproj_half(c + 1, 0)
                    qproj_half(c + 1, 1)
                # leftover O-proj tiles when T is too small to thread them all
                if c > 0 and t == T - 1 and i < NI and T - 2 < NMO:
                    for mo in range(max(T - 2, 0), NMO):
                        oproj_one(c - 1, mo)
                    oproj_flush(c - 1)
                # next scores, then the chunk-opening PVs dead last (they
                # are gated on the previous chunk's norm)
                if i + 1 < NI:
                    scores01(*divmod(i + 1, T))
                    scoresh2(*divmod(i + 1, T))
                for ch, th, h, jh in late_pv:
                    pv_one(ch, th, h)
                    if h == HPG - 1:
                        es_hist.pop(jh)
                    if th == T - 1:
                        norm_h(ch, h)
            # tail: keep the PE HAM-warm with scratch matmuls while the
            # last norms drain on DVE, then the last chunk's O-projection
            # in pairs - both WO0 matmuls (only need CTX01) ahead of the
            # WO2 matmuls (need norm_h2), evictions alternating ACT/DVE,
            # out-DMA per pair so the store overlaps the remaining compute
            for w in range(4):
                wps = ps_w.tile([P, 512], F32, tag="psw", name=f"tw{w}")
                nc.tensor.matmul(
                    wps, lhsT=WUP[:, 0:P], rhs=WUP, start=True, stop=True
                )
            lc = NQC - 1
            c0l = lc * QC
            for mo in range(0, NMO, 2):
                pos = []
                for k in range(2):
                    po = ps_w.tile(
                        [P, 512], F32, tag="psw", name=f"tl{mo + k}"
                    )
                    nc.tensor.matmul(
                        po, lhsT=WO0[:, (mo + k) * P:(mo + k + 1) * P],
                        rhs=CTX01[:, c0l:c0l + QC],
                        start=True, stop=False,
                    )
                    pos.append(po)
                for k in range(2):
                    nc.tensor.matmul(
                        pos[k], lhsT=WO2[:, (mo + k) * P:(mo + k + 1) * P],
                        rhs=CTX2[0:DH, c0l:c0l + QC],
                        start=False, stop=True,
                    )
                ot = osb_for(lc)
                for k in range(2):
                    dst = ot[:, (mo + k) * QC:(mo + k + 1) * QC]
                    if k == 0:
                        nc.scalar.activation(dst, pos[k], AFT.Identity, bias=0.0)
                    else:
                        nc.vector.tensor_copy(dst, pos[k])
                nc.sync.dma_start(
                    out=out[:, (lc * NMO + mo) * QC:(lc * NMO + mo + 2) * QC],
                    in_=ot[:, mo * QC:(mo + 2) * QC],
                )
            _osb.pop(lc)
    nc.compile()
    return nc


def _get_prog(KP):
    if KP not in _prog_cache:
        _prog_cache[KP] = _build_nc(KP)
    return _prog_cache[KP]


def _rearrange_w(Wslice, BF):
    # [768, 192] -> [128, 6*192] (p-major kt blocks), contiguous for DMA
    return np.ascontiguousarray(
        Wslice.reshape(DM // P, P, DQ).transpose(1, 0, 2).reshape(P, -1)
    ).astype(BF)


def _chunk_major(xT, KCH):
    # [768, KP] -> [128, NKT*KP] with per-partition layout [kc][kt][cols]
    x3 = xT.reshape(NKT, P, xT.shape[1])
    return np.concatenate(
        [
            np.ascontiguousarray(
                x3[:, :, c0:c0 + cw].transpose(1, 0, 2)
            ).reshape(P, NKT * cw)
            for c0, cw in KCH
        ],
        axis=1,
    )


def _run(inputs, trace=False):
    import ml_dtypes
    from concourse.bass_utils import run_bass_kernel_spmd

    BF = ml_dtypes.bfloat16

    query = np.asarray(inputs["query"], dtype=np.float32)
    key = np.asarray(inputs["key"], dtype=np.float32)
    value = np.asarray(inputs["value"], dtype=np.float32)
    mask = np.asarray(inputs["mask"])
    Wq = np.asarray(inputs["Wq"], dtype=np.float32)
    bq = np.asarray(inputs["bq"], dtype=np.float32)
    Wk = np.asarray(inputs["Wk"], dtype=np.float32)
    bk = np.asarray(inputs["bk"], dtype=np.float32)
    Wv = np.asarray(inputs["Wv"], dtype=np.float32)
    bv = np.asarray(inputs["bv"], dtype=np.float32)
    Wo = np.asarray(inputs["Wo"], dtype=np.float32)
    bo = np.asarray(inputs["bo"], dtype=np.float32)

    idx = [np.nonzero(mask[b, 0, 0] != 0)[0] for b in range(B)]
    keff = [len(i) for i in idx]
    KP = max(P, ((max(keff) + P - 1) // P) * P)
    T = KP // P
    KCH = [(o, min(512, KP - o)) for o in range(0, KP, 512)]

    nc = _get_prog(KP)

    per_batch = {}
    for b in range(B):
        # q: [128, NQC*NKT*QC] chunk-major ([c][kt][q])
        xqT = query[b].T.astype(BF)  # [768, 2048]
        xq_p = np.ascontiguousarray(
            xqT.reshape(NKT, P, NQC, QC).transpose(1, 2, 0, 3)
        ).reshape(P, -1)
        xkT = np.zeros((DM, KP), dtype=BF)
        xkT[:, :keff[b]] = key[b][idx[b]].T.astype(BF)
        xvT = np.zeros((DM, KP), dtype=BF)
        xvT[:, :keff[b]] = value[b][idx[b]].T.astype(BF)
        xk_p = _chunk_major(xkT, KCH)
        xv_p = _chunk_major(xvT, KCH)
        vmf = np.zeros((KP,), dtype=np.float32)
        vmf[:keff[b]] = 1.0
        vm2 = np.ascontiguousarray(vmf.reshape(T, P).T)  # [128, T]
        per_batch[b] = (xq_p, xk_p, xv_p, vm2)

    in_maps = []
    for core in range(NCORES):
        b, g = core // G, core % G
        xq_p, xk_p, xv_p, vm2 = per_batch[b]
        sl = slice(g * DQ, (g + 1) * DQ)
        wo_g = Wo[sl, :].astype(BF)  # [192, 768]
        wo_pack = np.concatenate(
            [wo_g[0:P], np.concatenate([wo_g[P:DQ], wo_g[P:DQ]], axis=0)],
            axis=1,
        )  # [128, 1536]
        mw = 4 + T + DQ
        msc = np.zeros((P, mw), dtype=np.float32)
        msc[:, 0] = bq[sl][0:P]
        msc[0:DH, 1] = bq[sl][P:DQ]
        msc[:, 2] = bk[sl][0:P]
        msc[0:DH, 3] = bk[sl][P:DQ]
        msc[:, 4:4 + T] = vm2
        msc[:, 4 + T:] = bv[sl][None, :]
        in_maps.append({
            "xq": xq_p,
            "xk": xk_p,
            "xv": xv_p,
            "wq": _rearrange_w(Wq[:, sl], BF),
            "wk": _rearrange_w(Wk[:, sl], BF),
            "wv": _rearrange_w(Wv[:, sl], BF),
            "wop": np.ascontiguousarray(wo_pack),
            "msc": msc,
        })

    res = run_bass_kernel_spmd(nc, in_maps, list(range(NCORES)), trace=trace)

    outp = np.zeros((B, S, DM), dtype=np.float32)
    for core in range(NCORES):
        o = np.asarray(res.results[core]["out"], dtype=np.float32)
        outT = o.reshape(P, NQC, NMO, QC).transpose(2, 0, 1, 3).reshape(DM, S)
        outp[core // G] += outT.T
    outp += bo.reshape(1, 1, DM)
    return outp, res


def kernel(**inputs) -> np.ndarray:
    out, _ = _run(inputs, trace=False)
    return out


if __name__ == "__main__":
    nc = _build_nc(1152)
    print("build OK")


# revision 50
# speedup vs baseline: 1.0892x; 1.0584x over previous
"""Multi-head attention (B=2, S=2048, d_model=768, H=12) on 8 TRN2 NeuronCores.

Sharding: 2-way data parallel over batch x 4-way tensor parallel over heads
(3 heads / 192-wide d_model slice per core). Host compacts masked keys away
(gather of unmasked key/value rows), pads to a 128 multiple, zero-fills pad
keys; softmax needs no mask handling on device (pad keys get V=0 and 0s in
the denominator ones-block). Host pre-arranges every input into the exact
SBUF layout (chunk-major, partition-contiguous) so each tensor loads as one
or two DMAs of 128 large packets; the output is likewise stored as one wide
DMA per query chunk and re-assembled on host.

Per core, a software-pipelined flat loop over (chunk c of 512 q, key tile t)
with emission order: exp01/exp2 (ACT) -> PV(i-lag) + norm (PE/DVE) ->
threaded projections (PE) -> scores(i+1) LAST. The PE queue therefore holds
pv+projection fill work ahead of scores(i+1), which is the only instruction
that has to wait for exp(i) (WAR on the scores PSUM tiles). sp is split into
sp01 (2 banks, heads 0/1, row-group-paired matmuls) and sp2 (1 bank, head 2)
so scores01(i+1) only waits on exp01(i). Steady-state period ~= the ACT busy
time per iteration.

V blocks of 128 cols/head, all [valid-ones 64 | V 64]: PV lands a
64-row-replicated denominator on partitions 0:64 and ctx on 64:128 of one
3-bank ctx PSUM tile (denominator costs no PE time - matmul cost scales
with N only). Normalization per head section: one wide fast reciprocal
straight from PSUM (partition base 0 only) -> psum*recip multiplies to bf16.
The first PV of each chunk trails two extra iterations so the previous
chunk's norm clears the ctx banks before the in-order PE reaches the
start=True PV.

Output projection runs transposed (outT[dm,q] = Wo_g^T @ ctx), in
adjacent-tile pairs so the two 64-contraction WO2 matmuls run in disjoint
PE row groups into different PSUM banks; bf16 results collect in one wide
SBUF tile per chunk and fly out as a single DMA. A burst of warm-up matmuls
on scratch data runs during the initial DMA wait so the PE HAM clock-gate
releases (1.2 -> 2.4 GHz) before the first real matmul arrives.
"""

import math

import numpy as np

B = 2
S = 2048
DM = 768
H = 12
DH = 64
G = 4              # head-group (tensor-parallel) degree
HPG = H // G       # heads per core
DQ = HPG * DH      # 192 d_model slice per core
NCORES = 8
P = 128
QC = 512           # query chunk
NQC = S // QC
NKT = DM // P      # 6 contraction tiles for projections
NMO = DM // P      # 6 output-projection tiles

_prog_cache = {}


def _build_nc(KP):
    import concourse.mybir as mybir
    import concourse.tile as tile
    from concourse import bacc

    F32 = mybir.dt.float32
    BF = mybir.dt.bfloat16
    AFT = mybir.ActivationFunctionType

    T = KP // P            # key tiles
    KCH = [(o, min(512, KP - o)) for o in range(0, KP, 512)]
    MW = 4 + T + DQ        # misc tensor cols: biases | vm | bv

    nc = bacc.Bacc(None, target_bir_lowering=False)
    # all inputs host-pre-arranged to [128, *] partition-contiguous layouts
    xq = nc.declare_dram_parameter("xq", [P, NQC * NKT * QC], BF, isOutput=False)
    xk = nc.declare_dram_parameter("xk", [P, NKT * KP], BF, isOutput=False)
    xv = nc.declare_dram_parameter("xv", [P, NKT * KP], BF, isOutput=False)
    wq = nc.declare_dram_parameter("wq", [P, NKT * DQ], BF, isOutput=False)
    wk = nc.declare_dram_parameter("wk", [P, NKT * DQ], BF, isOutput=False)
    wv = nc.declare_dram_parameter("wv", [P, NKT * DQ], BF, isOutput=False)
    wop = nc.declare_dram_parameter("wop", [P, 2 * DM], BF, isOutput=False)
    msc = nc.declare_dram_parameter("msc", [P, MW], F32, isOutput=False)
    out = nc.declare_dram_parameter("out", [P, NQC * NMO * QC], BF, isOutput=True)

    with tile.TileContext(nc) as tc:
        with (
            tc.tile_pool(name="persist", bufs=1) as persist,
            tc.tile_pool(name="es", bufs=6) as espool,
            tc.tile_pool(name="rc", bufs=4) as rcpool,
            tc.tile_pool(name="osb", bufs=4) as osb,
            tc.tile_pool(name="ps_sp", bufs=1, space="PSUM") as ps_sp,
            tc.tile_pool(name="ps_ctx", bufs=1, space="PSUM") as ps_ctx,
            tc.tile_pool(name="ps_w", bufs=2, space="PSUM") as ps_w,
        ):
            # ---- warm-up scratch + exp-table preload ----
            WUP = persist.tile([P, 512], BF, tag="WUP")
            nc.gpsimd.memset(WUP, 0.0)
            WRM = persist.tile([1, 2], F32, tag="WRM")
            nc.vector.memset(WRM, 0.0)
            nc.scalar.activation(WRM[:, 1:2], WRM[:, 0:1], AFT.Exp)
            for w in range(10):
                wps = ps_w.tile([P, 512], F32, tag="psw", name=f"warm{w}")
                nc.tensor.matmul(
                    wps, lhsT=WUP[:, 0:P], rhs=WUP, start=True, stop=True
                )

            # ---- constants / weights / activations (K path first) ----
            # DMA issue order tracks compute order: everything scores(0,0)
            # and exp(0,0) need first, then per-chunk pieces just in time.
            WK = persist.tile([P, NKT, DQ], BF, tag="WK")
            nc.sync.dma_start(out=WK, in_=wk[:, :].rearrange("p (kt m) -> p kt m", m=DQ))
            XKA = persist.tile([P, NKT * KP], BF, tag="XKA")
            c00, cw0 = KCH[0]
            h0w = (NKT // 2) * cw0
            nc.sync.dma_start(out=XKA[:, 0:h0w], in_=xk[:, 0:h0w])
            nc.sync.dma_start(out=XKA[:, h0w:NKT * cw0], in_=xk[:, h0w:NKT * cw0])
            MISC = persist.tile([P, MW], F32, tag="MISC")
            nc.sync.dma_start(out=MISC, in_=msc[:, :])
            WQ = persist.tile([P, NKT, DQ], BF, tag="WQ")
            nc.sync.dma_start(out=WQ, in_=wq[:, :].rearrange("p (kt m) -> p kt m", m=DQ))
            XQA = persist.tile([P, NQC * NKT * QC], BF, tag="XQA")
            nc.sync.dma_start(
                out=XQA[:, 0:NKT * QC], in_=xq[:, 0:NKT * QC]
            )
            if KP > cw0:
                nc.sync.dma_start(
                    out=XKA[:, NKT * cw0:], in_=xk[:, NKT * cw0:]
                )
            WV = persist.tile([P, NKT, DQ], BF, tag="WV")
            nc.sync.dma_start(out=WV, in_=wv[:, :].rearrange("p (kt m) -> p kt m", m=DQ))
            XVA = persist.tile([P, NKT * KP], BF, tag="XVA")
            nc.sync.dma_start(out=XVA[:, 0:NKT * cw0], in_=xv[:, 0:NKT * cw0])
            nc.sync.dma_start(
                out=XQA[:, NKT * QC:], in_=xq[:, NKT * QC:]
            )
            if KP > cw0:
                nc.sync.dma_start(
                    out=XVA[:, NKT * cw0:], in_=xv[:, NKT * cw0:]
                )
            WOP = persist.tile([P, 2 * DM], BF, tag="WOP")
            nc.sync.dma_start(out=WOP, in_=wop[:, :])

            # misc views
            BQ0 = MISC[:, 0:1]
            BQ1 = MISC[0:DH, 1:2]
            BK0 = MISC[:, 2:3]
            BK1 = MISC[0:DH, 3:4]
            VM = MISC[:, 4:4 + T]
            BV = MISC[:, 4 + T:4 + T + DQ]
            WO0 = WOP[:, 0:DM]          # wo rows 0:128 (h0,h1)
            WO2 = WOP[0:DH, DM:2 * DM]  # wo rows 128:192 (h2)

            # ---- persistent activations ----
            QT0 = persist.tile([P, S], BF, tag="QT0")    # heads 0,1
            QT1 = persist.tile([DH, S], BF, tag="QT1")   # head 2
            KT0 = persist.tile([P, KP], BF, tag="KT0")
            KT1 = persist.tile([DH, KP], BF, tag="KT1")  # head 2
            # V blocks of 128 cols per head (see module docstring)
            VP = persist.tile([P, T, HPG * P], BF, tag="VP")
            CTX01 = persist.tile([P, S], BF, tag="CTX01")  # h0 rows 0:64, h1 64:128
            CTX2 = persist.tile([DH, S], BF, tag="CTX2")   # h2

            ONES = persist.tile([P, HPG * DH], BF, tag="ONES")
            nc.vector.memset(ONES, 1.0)

            # ---- K projection (per key chunk; later chunks threaded into
            # the early attention iterations, DVE eviction) ----
            def kproj_m(kc, m, evict_on_act):
                c0, cw = KCH[kc]
                bias, mw = (BK0, P) if m == 0 else (BK1, DH)
                ps = ps_w.tile([P, 512], F32, tag="psw", name=f"kps{c0}_{m}")
                for kt in range(NKT):
                    nc.tensor.matmul(
                        ps[0:mw, 0:cw],
                        lhsT=WK[:, kt, m * P:m * P + mw],
                        rhs=XKA[:, c0 * NKT + kt * cw:c0 * NKT + (kt + 1) * cw],
                        start=(kt == 0),
                        stop=(kt == NKT - 1),
                    )
                dst = KT0 if m == 0 else KT1
                if evict_on_act:
                    nc.scalar.activation(
                        dst[0:mw, c0:c0 + cw], ps[0:mw, 0:cw],
                        AFT.Identity, bias=bias,
                    )
                else:
                    nc.vector.tensor_scalar_add(
                        dst[0:mw, c0:c0 + cw], ps[0:mw, 0:cw], bias
                    )

            def kproj_chunk(kc, evict_on_act):
                kproj_m(kc, 0, evict_on_act)
                kproj_m(kc, 1, evict_on_act)

            kproj_m(0, 0, True)

            _qps = {}

            def qproj_part(c, m, half, evict_on_act=False):
                c0 = c * QC
                bias, mw = (BQ0, P) if m == 0 else (BQ1, DH)
                if half == 0:
                    _qps[(c, m)] = ps_w.tile(
                        [P, 512], F32, tag="psw", name=f"qps{c}_{m}"
                    )
                ps = _qps[(c, m)]
                kts = range(0, NKT // 2) if half == 0 else range(NKT // 2, NKT)
                for kt in kts:
                    nc.tensor.matmul(
                        ps[0:mw, :],
                        lhsT=WQ[:, kt, m * P:m * P + mw],
                        rhs=XQA[:, (c * NKT + kt) * QC:(c * NKT + kt + 1) * QC],
                        start=(kt == 0),
                        stop=(kt == NKT - 1),
                    )
                if half == 0:
                    return
                dst = QT0 if m == 0 else QT1
                if evict_on_act:
                    nc.scalar.activation(
                        dst[0:mw, c0:c0 + QC], ps[0:mw, :], AFT.Identity, bias=bias
                    )
                else:
                    nc.vector.tensor_scalar_add(
                        dst[0:mw, c0:c0 + QC], ps[0:mw, :], bias
                    )
                del _qps[(c, m)]

            def qproj_half(c, m, evict_on_act=False):
                qproj_part(c, m, 0, evict_on_act)
                qproj_part(c, m, 1, evict_on_act)

            def vproj(t):
                kc = min(t * P // 512, len(KCH) - 1)
                c0, cw = KCH[kc]
                ps = ps_w.tile([P, 512], F32, tag="psw", name=f"vps{t}")
                for kt in range(NKT):
                    off = c0 * NKT + kt * cw + (t * P - c0)
                    nc.tensor.matmul(
                        ps[:, 0:DQ],
                        lhsT=XVA[:, off:off + P],
                        rhs=WV[:, kt, :],
                        start=(kt == 0),
                        stop=(kt == NKT - 1),
                    )
                # all blocks [valid-ones 0:64 | V 64:128]: PV rows 0:64 = den
                # (partition base 0 for the fused psum reciprocal), 64:128 ctx
                vview = VP[:, t, :].rearrange("p (h c) -> p h c", c=P)
                nc.vector.tensor_add(
                    vview[:, :, DH:P],
                    ps[:, 0:DQ].rearrange("p (h d) -> p h d", d=DH),
                    BV[:, :].rearrange("p (h d) -> p h d", d=DH),
                )
                nc.vector.tensor_scalar_mul(
                    vview[:, :, DH:P], vview[:, :, DH:P], VM[:, t:t + 1]
                )
                nc.vector.tensor_scalar_mul(
                    vview[:, :, 0:DH],
                    ONES[:, :].rearrange("p (h d) -> p h d", d=DH),
                    VM[:, t:t + 1],
                )

            # ---- attention, software-pipelined emission ----
            # sp split into two PSUM tiles so the WAR of scores(i+1) on
            # exp(i) releases per-section: scores01 only waits exp01.
            sp01 = ps_sp.tile([P, 1024], F32, tag="sp01")  # [h0 512 | h1 512]
            sp2 = ps_sp.tile([P, 512], F32, tag="sp2")     # h2
            # per-head ctx tiles: WAR tracking is per-tile, so head h's new
            # chunk only waits head h's norm, not the whole norm chain
            CTXP = [
                ps_ctx.tile([P, 512], F32, tag=f"ctx{h}", name=f"ctx{h}")
                for h in range(HPG)
            ]
            SCL = 1.0 / math.sqrt(DH)

            def scores01(c, t):
                c0 = c * QC
                tsl = slice(t * P, (t + 1) * P)
                nc.tensor.matmul(
                    sp01[:, 0:512],
                    lhsT=KT0[0:DH, tsl], rhs=QT0[0:DH, c0:c0 + QC],
                    start=True, stop=True,
                )
                nc.tensor.matmul(
                    sp01[:, 512:1024],
                    lhsT=KT0[DH:P, tsl], rhs=QT0[DH:P, c0:c0 + QC],
                    start=True, stop=True,
                )

            def scoresh2(c, t):
                c0 = c * QC
                tsl = slice(t * P, (t + 1) * P)
                nc.tensor.matmul(
                    sp2[:, 0:512],
                    lhsT=KT1[0:DH, tsl], rhs=QT1[0:DH, c0:c0 + QC],
                    start=True, stop=True,
                )

            # per-(chunk, tile-pair) out staging tiles: each pair gets its
            # own SBUF tile and its own out-DMA, so a store never WAR-blocks
            # the next pair's eviction
            _osb = {}

            def osb_for(c, pr):
                if (c, pr) not in _osb:
                    _osb[(c, pr)] = osb.tile(
                        [P, 2 * QC], BF, tag="posb", name=f"osb{c}_{pr}"
                    )
                return _osb[(c, pr)]

            def oproj_pair_flush(c, pr):
                nc.sync.dma_start(
                    out=out[:, (c * NMO + 2 * pr) * QC:(c * NMO + 2 * pr + 2) * QC],
                    in_=_osb.pop((c, pr)),
                )

            def oproj_one(c, mo, evict_act=False):
                # outT[mo*128:(mo+1)*128, chunk c] = WO0[:,mo].T@CTX01 + WO2[:,mo].T@CTX2
                c0 = c * QC
                po = ps_w.tile([P, 512], F32, tag="psw", name=f"po{c}_{mo}")
                nc.tensor.matmul(
                    po, lhsT=WO0[:, mo * P:(mo + 1) * P],
                    rhs=CTX01[:, c0:c0 + QC],
                    start=True, stop=False,
                )
                nc.tensor.matmul(
                    po, lhsT=WO2[:, mo * P:(mo + 1) * P],
                    rhs=CTX2[0:DH, c0:c0 + QC],
                    start=False, stop=True,
                )
                dst = osb_for(c, mo // 2)[:, (mo % 2) * QC:(mo % 2 + 1) * QC]
                if evict_act:
                    nc.scalar.activation(dst, po, AFT.Identity, bias=0.0)
                else:
                    nc.vector.tensor_copy(dst, po)
                if mo % 2 == 1:
                    oproj_pair_flush(c, mo // 2)

            def pv_one(c, t, h):
                e01, e2 = es_hist[c * T + t]
                rhs = e01[:, h * 512:(h + 1) * 512] if h < 2 else e2[:, 0:512]
                nc.tensor.matmul(
                    CTXP[h],
                    lhsT=VP[:, t, h * P:(h + 1) * P],
                    rhs=rhs,
                    start=(t == 0), stop=(t == T - 1),
                )

            _NRM = [
                (CTX01, slice(0, DH)),
                (CTX01, slice(DH, P)),
                (CTX2, slice(0, DH)),
            ]

            def norm_h(c, h):
                # den rows 0:64 of this head's ctx tile (partition base 0):
                # fused reciprocal psum->sbuf, then one multiply.
                c0 = c * QC
                rc = rcpool.tile([DH, 512], F32, tag="rc", name=f"rc{c}_{h}")
                nc.vector.reciprocal_approx_fast(rc, CTXP[h][0:DH, :])
                dst, dr = _NRM[h]
                nc.vector.tensor_mul(
                    dst[dr, c0:c0 + QC], CTXP[h][DH:P, :], rc
                )

            # Flat pipeline over i = c*T + t. Emission order per iteration:
            # exp01(i), exp2(i) [ACT] -> PV + per-head norm [PE/DVE] ->
            # threaded projections [PE] -> scores(i+1) -> chunk-opening PVs
            # LAST. Head h's PV trails by 1+h iterations (head skew): each
            # head finishes its chunk - and its norm slice - on a different
            # iteration, so normalization and the next chunk's start=True
            # PVs overlap the boundary instead of serializing behind one big
            # norm. t==0 PVs (which WAR the previous chunk's norm) go after
            # scores so they never head-of-line block the PE queue.
            NI = NQC * T
            es_hist = {}
            # interleave the startup so scores(0,0) - and with it the exp
            # pipeline - starts as soon as the m0 halves of K/Q are ready;
            # kproj m1 runs before qproj, and a second burst of scratch
            # matmuls bridges the wait for the first q chunk so the PE
            # never idles into a HAM re-throttle
            kproj_m(0, 1, True)
            for w in range(8):
                wps = ps_w.tile([P, 512], F32, tag="psw", name=f"warmb{w}")
                nc.tensor.matmul(
                    wps, lhsT=WUP[:, 0:P], rhs=WUP, start=True, stop=True
                )
            qproj_half(0, 0, evict_on_act=True)
            scores01(0, 0)
            qproj_half(0, 1, evict_on_act=True)
            scoresh2(0, 0)
            for i in range(NI + 1 + HPG - 1):
                c, t = divmod(min(i, NI - 1), T)
                if i < NI:
                    e01 = espool.tile(
                        [P, 1024], BF, tag="es01", name=f"es01_{c}_{t}"
                    )
                    e2 = espool.tile(
                        [P, 512], BF, tag="es2", name=f"es2_{c}_{t}"
                    )
                    es_hist[c * T + t] = (e01, e2)
                    nc.scalar.activation(
                        e01, sp01, AFT.Exp, bias=0.0, scale=SCL
                    )
                    nc.scalar.activation(
                        e2, sp2, AFT.Exp, bias=0.0, scale=SCL
                    )
                if i >= NI:
                    # drain iterations: scratch matmuls keep the HAM clock
                    # released while the final norms run on DVE
                    for w in range(4):
                        wps = ps_w.tile(
                            [P, 512], F32, tag="psw", name=f"dw{i}_{w}"
                        )
                        nc.tensor.matmul(
                            wps, lhsT=WUP[:, 0:P], rhs=WUP,
                            start=True, stop=True,
                        )
                late_pv = []
                norms_due = []
                for h in range(HPG):
                    jh = i - 1 - h
                    if not (0 <= jh < NI):
                        continue
                    ch, th = divmod(jh, T)
                    if th == 0 and ch > 0:
                        late_pv.append((ch, th, h, jh))
                        continue
                    pv_one(ch, th, h)
                    if h == HPG - 1:
                        es_hist.pop(jh)
                    if th == T - 1:
                        norms_due.append((ch, h))
                for ch, h in norms_due:
                    norm_h(ch, h)
                # threaded non-attention work, keyed by the exp index (c, t)
                if c == 0 and i < NI and t + 1 < len(KCH):
                    kproj_chunk(t + 1, False)
                if c == 0 and i < NI:
                    if t == 0:
                        vproj(0)
                        if T > 1:
                            vproj(1)
                    elif t + 1 < T:
                        vproj(t + 1)
                if c > 0 and i < NI and 2 <= t <= min(T - 1, NMO + 1):
                    oproj_one(c - 1, t - 2)
                if c + 1 < NQC and T >= 3 and i < NI:
                    if t == T - 3:
                        qproj_half(c + 1, 0)
                    elif t == T - 2:
                        qproj_half(c + 1, 1)
                if c + 1 < NQC and T < 3 and i < NI and t == T - 1:
                    qproj_half(c + 1, 0)
                    qproj_half(c + 1, 1)
                # leftover O-proj tiles when T is too small to thread them all
                if c > 0 and t == T - 1 and i < NI and T - 2 < NMO:
                    for mo in range(max(T - 2, 0), NMO):
                        oproj_one(c - 1, mo)
                # next scores, then the chunk-opening PVs dead last (they
                # are gated on the previous chunk's norm)
                if i + 1 < NI:
                    scores01(*divmod(i + 1, T))
                    scoresh2(*divmod(i + 1, T))
                for ch, th, h, jh in late_pv:
                    pv_one(ch, th, h)
                    if h == HPG - 1:
                        es_hist.pop(jh)
                    if th == T - 1:
                        norm_h(ch, h)
            # tail: keep the PE HAM-warm with scratch matmuls while the
            # last norms drain on DVE, then the last chunk's O-projection
            # in pairs - both WO0 matmuls (only need CTX01) ahead of the
            # WO2 matmuls (need norm_h2), evictions alternating ACT/DVE,
            # out-DMA per pair so the store overlaps the remaining compute
            for w in range(4):
                wps = ps_w.tile([P, 512], F32, tag="psw", name=f"tw{w}")
                nc.tensor.matmul(
                    wps, lhsT=WUP[:, 0:P], rhs=WUP, start=True, stop=True
                )
            lc = NQC - 1
            c0l = lc * QC
            for mo in range(0, NMO, 2):
                pos = []
                for k in range(2):
                    po = ps_w.tile(
                        [P, 512], F32, tag="psw", name=f"tl{mo + k}"
                    )
                    nc.tensor.matmul(
                        po, lhsT=WO0[:, (mo + k) * P:(mo + k + 1) * P],
                        rhs=CTX01[:, c0l:c0l + QC],
                        start=True, stop=False,
                    )
                    pos.append(po)
                for k in range(2):
                    nc.tensor.matmul(
                        pos[k], lhsT=WO2[:, (mo + k) * P:(mo + k + 1) * P],
                        rhs=CTX2[0:DH, c0l:c0l + QC],
                        start=False, stop=True,
                    )
                ot = osb_for(lc, mo // 2)
                for k in range(2):
                    dst = ot[:, k * QC:(k + 1) * QC]
                    if k == 0:
                        nc.scalar.activation(dst, pos[k], AFT.Identity, bias=0.0)
                    else:
                        nc.vector.tensor_copy(dst, pos[k])
                oproj_pair_flush(lc, mo // 2)
    nc.compile()
    return nc


def _get_prog(KP):
    if KP not in _prog_cache:
        _prog_cache[KP] = _build_nc(KP)
    return _prog_cache[KP]


def _rearrange_w(Wslice, BF):
    # [768, 192] -> [128, 6*192] (p-major kt blocks), contiguous for DMA
    return np.ascontiguousarray(
        Wslice.reshape(DM // P, P, DQ).transpose(1, 0, 2).reshape(P, -1)
    ).astype(BF)


def _chunk_major(xT, KCH):
    # [768, KP] -> [128, NKT*KP] with per-partition layout [kc][kt][cols]
    x3 = xT.reshape(NKT, P, xT.shape[1])
    return np.concatenate(
        [
            np.ascontiguousarray(
                x3[:, :, c0:c0 + cw].transpose(1, 0, 2)
            ).reshape(P, NKT * cw)
            for c0, cw in KCH
        ],
        axis=1,
    )


def _run(inputs, trace=False):
    import ml_dtypes
    from concourse.bass_utils import run_bass_kernel_spmd

    BF = ml_dtypes.bfloat16

    query = np.asarray(inputs["query"], dtype=np.float32)
    key = np.asarray(inputs["key"], dtype=np.float32)
    value = np.asarray(inputs["value"], dtype=np.float32)
    mask = np.asarray(inputs["mask"])
    Wq = np.asarray(inputs["Wq"], dtype=np.float32)
    bq = np.asarray(inputs["bq"], dtype=np.float32)
    Wk = np.asarray(inputs["Wk"], dtype=np.float32)
    bk = np.asarray(inputs["bk"], dtype=np.float32)
    Wv = np.asarray(inputs["Wv"], dtype=np.float32)
    bv = np.asarray(inputs["bv"], dtype=np.float32)
    Wo = np.asarray(inputs["Wo"], dtype=np.float32)
    bo = np.asarray(inputs["bo"], dtype=np.float32)

    idx = [np.nonzero(mask[b, 0, 0] != 0)[0] for b in range(B)]
    keff = [len(i) for i in idx]
    KP = max(P, ((max(keff) + P - 1) // P) * P)
    T = KP // P
    KCH = [(o, min(512, KP - o)) for o in range(0, KP, 512)]

    nc = _get_prog(KP)

    per_batch = {}
    for b in range(B):
        # q: [128, NQC*NKT*QC] chunk-major ([c][kt][q])
        xqT = query[b].T.astype(BF)  # [768, 2048]
        xq_p = np.ascontiguousarray(
            xqT.reshape(NKT, P, NQC, QC).transpose(1, 2, 0, 3)
        ).reshape(P, -1)
        xkT = np.zeros((DM, KP), dtype=BF)
        xkT[:, :keff[b]] = key[b][idx[b]].T.astype(BF)
        xvT = np.zeros((DM, KP), dtype=BF)
        xvT[:, :keff[b]] = value[b][idx[b]].T.astype(BF)
        xk_p = _chunk_major(xkT, KCH)
        xv_p = _chunk_major(xvT, KCH)
        vmf = np.zeros((KP,), dtype=np.float32)
        vmf[:keff[b]] = 1.0
        vm2 = np.ascontiguousarray(vmf.reshape(T, P).T)  # [128, T]
        per_batch[b] = (xq_p, xk_p, xv_p, vm2)

    in_maps = []
    for core in range(NCORES):
        b, g = core // G, core % G
        xq_p, xk_p, xv_p, vm2 = per_batch[b]
        sl = slice(g * DQ, (g + 1) * DQ)
        wo_g = Wo[sl, :].astype(BF)  # [192, 768]
        wo_pack = np.concatenate(
            [wo_g[0:P], np.concatenate([wo_g[P:DQ], wo_g[P:DQ]], axis=0)],
            axis=1,
        )  # [128, 1536]
        mw = 4 + T + DQ
        msc = np.zeros((P, mw), dtype=np.float32)
        msc[:, 0] = bq[sl][0:P]
        msc[0:DH, 1] = bq[sl][P:DQ]
        msc[:, 2] = bk[sl][0:P]
        msc[0:DH, 3] = bk[sl][P:DQ]
        msc[:, 4:4 + T] = vm2
        msc[:, 4 + T:] = bv[sl][None, :]
        in_maps.append({
            "xq": xq_p,
            "xk": xk_p,
            "xv": xv_p,
            "wq": _rearrange_w(Wq[:, sl], BF),
            "wk": _rearrange_w(Wk[:, sl], BF),
            "wv": _rearrange_w(Wv[:, sl], BF),
            "wop": np.ascontiguousarray(wo_pack),
            "msc": msc,
        })

    res = run_bass_kernel_spmd(nc, in_maps, list(range(NCORES)), trace=trace)

    outp = np.zeros((B, S, DM), dtype=np.float32)
    for core in range(NCORES):
        o = np.asarray(res.results[core]["out"], dtype=np.float32)
        outT = o.reshape(P, NQC, NMO, QC).transpose(2, 0, 1, 3).reshape(DM, S)
        outp[core // G] += outT.T
    outp += bo.reshape(1, 1, DM)
    return outp, res


def kernel(**inputs) -> np.ndarray:
    out, _ = _run(inputs, trace=False)
    return out


if __name__ == "__main__":
    nc = _build_nc(1152)
    print("build OK")


# revision 51
# speedup vs baseline: 1.0950x; 1.0054x over previous
"""Multi-head attention (B=2, S=2048, d_model=768, H=12) on 8 TRN2 NeuronCores.

Sharding: 2-way data parallel over batch x 4-way tensor parallel over heads
(3 heads / 192-wide d_model slice per core). Host compacts masked keys away
(gather of unmasked key/value rows), pads to a 128 multiple, zero-fills pad
keys; softmax needs no mask handling on device (pad keys get V=0 and 0s in
the denominator ones-block). Host pre-arranges every input into the exact
SBUF layout (chunk-major, partition-contiguous) so each tensor loads as one
or two DMAs of 128 large packets; the output is likewise stored as one wide
DMA per query chunk and re-assembled on host.

Per core, a software-pipelined flat loop over (chunk c of 512 q, key tile t)
with emission order: exp01/exp2 (ACT) -> PV(i-lag) + norm (PE/DVE) ->
threaded projections (PE) -> scores(i+1) LAST. The PE queue therefore holds
pv+projection fill work ahead of scores(i+1), which is the only instruction
that has to wait for exp(i) (WAR on the scores PSUM tiles). sp is split into
sp01 (2 banks, heads 0/1, row-group-paired matmuls) and sp2 (1 bank, head 2)
so scores01(i+1) only waits on exp01(i). Steady-state period ~= the ACT busy
time per iteration.

V blocks of 128 cols/head, all [valid-ones 64 | V 64]: PV lands a
64-row-replicated denominator on partitions 0:64 and ctx on 64:128 of one
3-bank ctx PSUM tile (denominator costs no PE time - matmul cost scales
with N only). Normalization per head section: one wide fast reciprocal
straight from PSUM (partition base 0 only) -> psum*recip multiplies to bf16.
The first PV of each chunk trails two extra iterations so the previous
chunk's norm clears the ctx banks before the in-order PE reaches the
start=True PV.

Output projection runs transposed (outT[dm,q] = Wo_g^T @ ctx), in
adjacent-tile pairs so the two 64-contraction WO2 matmuls run in disjoint
PE row groups into different PSUM banks; bf16 results collect in one wide
SBUF tile per chunk and fly out as a single DMA. A burst of warm-up matmuls
on scratch data runs during the initial DMA wait so the PE HAM clock-gate
releases (1.2 -> 2.4 GHz) before the first real matmul arrives.
"""

import math

import numpy as np

B = 2
S = 2048
DM = 768
H = 12
DH = 64
G = 4              # head-group (tensor-parallel) degree
HPG = H // G       # heads per core
DQ = HPG * DH      # 192 d_model slice per core
NCORES = 8
P = 128
QC = 512           # query chunk
NQC = S // QC
NKT = DM // P      # 6 contraction tiles for projections
NMO = DM // P      # 6 output-projection tiles

_prog_cache = {}


def _build_nc(KP):
    import concourse.mybir as mybir
    import concourse.tile as tile
    from concourse import bacc

    F32 = mybir.dt.float32
    BF = mybir.dt.bfloat16
    AFT = mybir.ActivationFunctionType

    T = KP // P            # key tiles
    KCH = [(o, min(512, KP - o)) for o in range(0, KP, 512)]
    MW = 4 + T + DQ        # misc tensor cols: biases | vm | bv

    nc = bacc.Bacc(None, target_bir_lowering=False)
    # all inputs host-pre-arranged to [128, *] partition-contiguous layouts
    xq = nc.declare_dram_parameter("xq", [P, NQC * NKT * QC], BF, isOutput=False)
    xk = nc.declare_dram_parameter("xk", [P, NKT * KP], BF, isOutput=False)
    xv = nc.declare_dram_parameter("xv", [P, NKT * KP], BF, isOutput=False)
    wq = nc.declare_dram_parameter("wq", [P, NKT * DQ], BF, isOutput=False)
    wk = nc.declare_dram_parameter("wk", [P, NKT * DQ], BF, isOutput=False)
    wv = nc.declare_dram_parameter("wv", [P, NKT * DQ], BF, isOutput=False)
    wop = nc.declare_dram_parameter("wop", [P, 2 * DM], BF, isOutput=False)
    msc = nc.declare_dram_parameter("msc", [P, MW], F32, isOutput=False)
    out = nc.declare_dram_parameter("out", [P, NQC * NMO * QC], BF, isOutput=True)

    with tile.TileContext(nc) as tc:
        with (
            tc.tile_pool(name="persist", bufs=1) as persist,
            tc.tile_pool(name="es", bufs=6) as espool,
            tc.tile_pool(name="rc", bufs=4) as rcpool,
            tc.tile_pool(name="osb", bufs=4) as osb,
            tc.tile_pool(name="ps_sp", bufs=1, space="PSUM") as ps_sp,
            tc.tile_pool(name="ps_ctx", bufs=1, space="PSUM") as ps_ctx,
            tc.tile_pool(name="ps_w", bufs=2, space="PSUM") as ps_w,
        ):
            # ---- warm-up scratch + exp-table preload ----
            WUP = persist.tile([P, 512], BF, tag="WUP")
            nc.gpsimd.memset(WUP, 0.0)
            WRM = persist.tile([1, 2], F32, tag="WRM")
            nc.vector.memset(WRM, 0.0)
            nc.scalar.activation(WRM[:, 1:2], WRM[:, 0:1], AFT.Exp)
            for w in range(10):
                wps = ps_w.tile([P, 512], F32, tag="psw", name=f"warm{w}")
                nc.tensor.matmul(
                    wps, lhsT=WUP[:, 0:P], rhs=WUP, start=True, stop=True
                )

            # ---- constants / weights / activations (K path first) ----
            # DMA issue order tracks compute order: everything scores(0,0)
            # and exp(0,0) need first, then per-chunk pieces just in time.
            WK = persist.tile([P, NKT, DQ], BF, tag="WK")
            nc.sync.dma_start(out=WK, in_=wk[:, :].rearrange("p (kt m) -> p kt m", m=DQ))
            XKA = persist.tile([P, NKT * KP], BF, tag="XKA")
            c00, cw0 = KCH[0]
            h0w = (NKT // 2) * cw0
            nc.sync.dma_start(out=XKA[:, 0:h0w], in_=xk[:, 0:h0w])
            nc.sync.dma_start(out=XKA[:, h0w:NKT * cw0], in_=xk[:, h0w:NKT * cw0])
            MISC = persist.tile([P, MW], F32, tag="MISC")
            nc.sync.dma_start(out=MISC, in_=msc[:, :])
            WQ = persist.tile([P, NKT, DQ], BF, tag="WQ")
            nc.sync.dma_start(out=WQ, in_=wq[:, :].rearrange("p (kt m) -> p kt m", m=DQ))
            XQA = persist.tile([P, NQC * NKT * QC], BF, tag="XQA")
            nc.sync.dma_start(
                out=XQA[:, 0:NKT * QC], in_=xq[:, 0:NKT * QC]
            )
            if KP > cw0:
                nc.sync.dma_start(
                    out=XKA[:, NKT * cw0:], in_=xk[:, NKT * cw0:]
                )
            WV = persist.tile([P, NKT, DQ], BF, tag="WV")
            nc.sync.dma_start(out=WV, in_=wv[:, :].rearrange("p (kt m) -> p kt m", m=DQ))
            XVA = persist.tile([P, NKT * KP], BF, tag="XVA")
            nc.sync.dma_start(out=XVA[:, 0:NKT * cw0], in_=xv[:, 0:NKT * cw0])
            if KP > cw0:
                nc.sync.dma_start(
                    out=XVA[:, NKT * cw0:], in_=xv[:, NKT * cw0:]
                )
            for qc in range(1, NQC):
                nc.sync.dma_start(
                    out=XQA[:, qc * NKT * QC:(qc + 1) * NKT * QC],
                    in_=xq[:, qc * NKT * QC:(qc + 1) * NKT * QC],
                )
            WOP = persist.tile([P, 2 * DM], BF, tag="WOP")
            nc.sync.dma_start(out=WOP, in_=wop[:, :])

            # misc views
            BQ0 = MISC[:, 0:1]
            BQ1 = MISC[0:DH, 1:2]
            BK0 = MISC[:, 2:3]
            BK1 = MISC[0:DH, 3:4]
            VM = MISC[:, 4:4 + T]
            BV = MISC[:, 4 + T:4 + T + DQ]
            WO0 = WOP[:, 0:DM]          # wo rows 0:128 (h0,h1)
            WO2 = WOP[0:DH, DM:2 * DM]  # wo rows 128:192 (h2)

            # ---- persistent activations ----
            QT0 = persist.tile([P, S], BF, tag="QT0")    # heads 0,1
            QT1 = persist.tile([DH, S], BF, tag="QT1")   # head 2
            KT0 = persist.tile([P, KP], BF, tag="KT0")
            KT1 = persist.tile([DH, KP], BF, tag="KT1")  # head 2
            # V blocks of 128 cols per head (see module docstring)
            VP = persist.tile([P, T, HPG * P], BF, tag="VP")
            CTX01 = persist.tile([P, S], BF, tag="CTX01")  # h0 rows 0:64, h1 64:128
            CTX2 = persist.tile([DH, S], BF, tag="CTX2")   # h2

            ONES = persist.tile([P, HPG * DH], BF, tag="ONES")
            nc.vector.memset(ONES, 1.0)

            # ---- K projection (per key chunk; later chunks threaded into
            # the early attention iterations, DVE eviction) ----
            def kproj_m(kc, m, evict_on_act):
                c0, cw = KCH[kc]
                bias, mw = (BK0, P) if m == 0 else (BK1, DH)
                ps = ps_w.tile([P, 512], F32, tag="psw", name=f"kps{c0}_{m}")
                for kt in range(NKT):
                    nc.tensor.matmul(
                        ps[0:mw, 0:cw],
                        lhsT=WK[:, kt, m * P:m * P + mw],
                        rhs=XKA[:, c0 * NKT + kt * cw:c0 * NKT + (kt + 1) * cw],
                        start=(kt == 0),
                        stop=(kt == NKT - 1),
                    )
                dst = KT0 if m == 0 else KT1
                if evict_on_act:
                    nc.scalar.activation(
                        dst[0:mw, c0:c0 + cw], ps[0:mw, 0:cw],
                        AFT.Identity, bias=bias,
                    )
                else:
                    nc.vector.tensor_scalar_add(
                        dst[0:mw, c0:c0 + cw], ps[0:mw, 0:cw], bias
                    )

            def kproj_chunk(kc, evict_on_act):
                kproj_m(kc, 0, evict_on_act)
                kproj_m(kc, 1, evict_on_act)

            kproj_m(0, 0, True)

            _qps = {}

            def qproj_part(c, m, half, evict_on_act=False):
                c0 = c * QC
                bias, mw = (BQ0, P) if m == 0 else (BQ1, DH)
                if half == 0:
                    _qps[(c, m)] = ps_w.tile(
                        [P, 512], F32, tag="psw", name=f"qps{c}_{m}"
                    )
                ps = _qps[(c, m)]
                kts = range(0, NKT // 2) if half == 0 else range(NKT // 2, NKT)
                for kt in kts:
                    nc.tensor.matmul(
                        ps[0:mw, :],
                        lhsT=WQ[:, kt, m * P:m * P + mw],
                        rhs=XQA[:, (c * NKT + kt) * QC:(c * NKT + kt + 1) * QC],
                        start=(kt == 0),
                        stop=(kt == NKT - 1),
                    )
                if half == 0:
                    return
                dst = QT0 if m == 0 else QT1
                if evict_on_act:
                    nc.scalar.activation(
                        dst[0:mw, c0:c0 + QC], ps[0:mw, :], AFT.Identity, bias=bias
                    )
                else:
                    nc.vector.tensor_scalar_add(
                        dst[0:mw, c0:c0 + QC], ps[0:mw, :], bias
                    )
                del _qps[(c, m)]

            def qproj_half(c, m, evict_on_act=False):
                qproj_part(c, m, 0, evict_on_act)
                qproj_part(c, m, 1, evict_on_act)

            def vproj(t):
                kc = min(t * P // 512, len(KCH) - 1)
                c0, cw = KCH[kc]
                ps = ps_w.tile([P, 512], F32, tag="psw", name=f"vps{t}")
                for kt in range(NKT):
                    off = c0 * NKT + kt * cw + (t * P - c0)
                    nc.tensor.matmul(
                        ps[:, 0:DQ],
                        lhsT=XVA[:, off:off + P],
                        rhs=WV[:, kt, :],
                        start=(kt == 0),
                        stop=(kt == NKT - 1),
                    )
                # all blocks [valid-ones 0:64 | V 64:128]: PV rows 0:64 = den
                # (partition base 0 for the fused psum reciprocal), 64:128 ctx
                vview = VP[:, t, :].rearrange("p (h c) -> p h c", c=P)
                nc.vector.tensor_add(
                    vview[:, :, DH:P],
                    ps[:, 0:DQ].rearrange("p (h d) -> p h d", d=DH),
                    BV[:, :].rearrange("p (h d) -> p h d", d=DH),
                )
                nc.vector.tensor_scalar_mul(
                    vview[:, :, DH:P], vview[:, :, DH:P], VM[:, t:t + 1]
                )
                nc.vector.tensor_scalar_mul(
                    vview[:, :, 0:DH],
                    ONES[:, :].rearrange("p (h d) -> p h d", d=DH),
                    VM[:, t:t + 1],
                )

            # ---- attention, software-pipelined emission ----
            # sp split into two PSUM tiles so the WAR of scores(i+1) on
            # exp(i) releases per-section: scores01 only waits exp01.
            sp01 = ps_sp.tile([P, 1024], F32, tag="sp01")  # [h0 512 | h1 512]
            sp2 = ps_sp.tile([P, 512], F32, tag="sp2")     # h2
            # per-head ctx tiles: WAR tracking is per-tile, so head h's new
            # chunk only waits head h's norm, not the whole norm chain
            CTXP = [
                ps_ctx.tile([P, 512], F32, tag=f"ctx{h}", name=f"ctx{h}")
                for h in range(HPG)
            ]
            SCL = 1.0 / math.sqrt(DH)

            def scores01(c, t):
                c0 = c * QC
                tsl = slice(t * P, (t + 1) * P)
                nc.tensor.matmul(
                    sp01[:, 0:512],
                    lhsT=KT0[0:DH, tsl], rhs=QT0[0:DH, c0:c0 + QC],
                    start=True, stop=True,
                )
                nc.tensor.matmul(
                    sp01[:, 512:1024],
                    lhsT=KT0[DH:P, tsl], rhs=QT0[DH:P, c0:c0 + QC],
                    start=True, stop=True,
                )

            def scoresh2(c, t):
                c0 = c * QC
                tsl = slice(t * P, (t + 1) * P)
                nc.tensor.matmul(
                    sp2[:, 0:512],
                    lhsT=KT1[0:DH, tsl], rhs=QT1[0:DH, c0:c0 + QC],
                    start=True, stop=True,
                )

            # per-(chunk, tile-pair) out staging tiles: each pair gets its
            # own SBUF tile and its own out-DMA, so a store never WAR-blocks
            # the next pair's eviction
            _osb = {}

            def osb_for(c, pr):
                if (c, pr) not in _osb:
                    _osb[(c, pr)] = osb.tile(
                        [P, 2 * QC], BF, tag="posb", name=f"osb{c}_{pr}"
                    )
                return _osb[(c, pr)]

            def oproj_pair_flush(c, pr):
                nc.sync.dma_start(
                    out=out[:, (c * NMO + 2 * pr) * QC:(c * NMO + 2 * pr + 2) * QC],
                    in_=_osb.pop((c, pr)),
                )

            def oproj_one(c, mo, evict_act=False):
                # outT[mo*128:(mo+1)*128, chunk c] = WO0[:,mo].T@CTX01 + WO2[:,mo].T@CTX2
                c0 = c * QC
                po = ps_w.tile([P, 512], F32, tag="psw", name=f"po{c}_{mo}")
                nc.tensor.matmul(
                    po, lhsT=WO0[:, mo * P:(mo + 1) * P],
                    rhs=CTX01[:, c0:c0 + QC],
                    start=True, stop=False,
                )
                nc.tensor.matmul(
                    po, lhsT=WO2[:, mo * P:(mo + 1) * P],
                    rhs=CTX2[0:DH, c0:c0 + QC],
                    start=False, stop=True,
                )
                dst = osb_for(c, mo // 2)[:, (mo % 2) * QC:(mo % 2 + 1) * QC]
                if evict_act:
                    nc.scalar.activation(dst, po, AFT.Identity, bias=0.0)
                else:
                    nc.vector.tensor_copy(dst, po)
                if mo % 2 == 1:
                    oproj_pair_flush(c, mo // 2)

            def pv_one(c, t, h):
                e01, e2 = es_hist[c * T + t]
                rhs = e01[:, h * 512:(h + 1) * 512] if h < 2 else e2[:, 0:512]
                nc.tensor.matmul(
                    CTXP[h],
                    lhsT=VP[:, t, h * P:(h + 1) * P],
                    rhs=rhs,
                    start=(t == 0), stop=(t == T - 1),
                )

            _NRM = [
                (CTX01, slice(0, DH)),
                (CTX01, slice(DH, P)),
                (CTX2, slice(0, DH)),
            ]

            def norm_h(c, h):
                # den rows 0:64 of this head's ctx tile (partition base 0):
                # fused reciprocal psum->sbuf, then one multiply.
                c0 = c * QC
                rc = rcpool.tile([DH, 512], F32, tag="rc", name=f"rc{c}_{h}")
                nc.vector.reciprocal_approx_fast(rc, CTXP[h][0:DH, :])
                dst, dr = _NRM[h]
                nc.vector.tensor_mul(
                    dst[dr, c0:c0 + QC], CTXP[h][DH:P, :], rc
                )

            # Flat pipeline over i = c*T + t. Emission order per iteration:
            # exp01(i), exp2(i) [ACT] -> PV + per-head norm [PE/DVE] ->
            # threaded projections [PE] -> scores(i+1) -> chunk-opening PVs
            # LAST. Head h's PV trails by 1+h iterations (head skew): each
            # head finishes its chunk - and its norm slice - on a different
            # iteration, so normalization and the next chunk's start=True
            # PVs overlap the boundary instead of serializing behind one big
            # norm. t==0 PVs (which WAR the previous chunk's norm) go after
            # scores so they never head-of-line block the PE queue.
            NI = NQC * T
            es_hist = {}
            # interleave the startup so scores(0,0) - and with it the exp
            # pipeline - starts as soon as the m0 halves of K/Q are ready;
            # kproj m1 runs before qproj, and a second burst of scratch
            # matmuls bridges the wait for the first q chunk so the PE
            # never idles into a HAM re-throttle
            kproj_m(0, 1, True)
            for w in range(8):
                wps = ps_w.tile([P, 512], F32, tag="psw", name=f"warmb{w}")
                nc.tensor.matmul(
                    wps, lhsT=WUP[:, 0:P], rhs=WUP, start=True, stop=True
                )
            qproj_half(0, 0, evict_on_act=True)
            scores01(0, 0)
            qproj_half(0, 1, evict_on_act=True)
            scoresh2(0, 0)
            for i in range(NI + 1 + HPG - 1):
                c, t = divmod(min(i, NI - 1), T)
                if i < NI:
                    e01 = espool.tile(
                        [P, 1024], BF, tag="es01", name=f"es01_{c}_{t}"
                    )
                    e2 = espool.tile(
                        [P, 512], BF, tag="es2", name=f"es2_{c}_{t}"
                    )
                    es_hist[c * T + t] = (e01, e2)
                    nc.scalar.activation(
                        e01, sp01, AFT.Exp, bias=0.0, scale=SCL
                    )
                    nc.scalar.activation(
                        e2, sp2, AFT.Exp, bias=0.0, scale=SCL
                    )
                if i >= NI:
                    # drain iterations: scratch matmuls keep the HAM clock
                    # released while the final norms run on DVE
                    for w in range(4):
                        wps = ps_w.tile(
                            [P, 512], F32, tag="psw", name=f"dw{i}_{w}"
                        )
                        nc.tensor.matmul(
                            wps, lhsT=WUP[:, 0:P], rhs=WUP,
                            start=True, stop=True,
                        )
                late_pv = []
                norms_due = []
                for h in range(HPG):
                    jh = i - 1 - h
                    if not (0 <= jh < NI):
                        continue
                    ch, th = divmod(jh, T)
                    if th == 0 and ch > 0:
                        late_pv.append((ch, th, h, jh))
                        continue
                    pv_one(ch, th, h)
                    if h == HPG - 1:
                        es_hist.pop(jh)
                    if th == T - 1:
                        norms_due.append((ch, h))
                for ch, h in norms_due:
                    norm_h(ch, h)
                # threaded non-attention work, keyed by the exp index (c, t)
                if c == 0 and i < NI and t + 1 < len(KCH):
                    kproj_chunk(t + 1, False)
                if c == 0 and i < NI:
                    if t == 0:
                        vproj(0)
                        if T > 1:
                            vproj(1)
                    elif t + 1 < T:
                        vproj(t + 1)
                if c > 0 and i < NI and 2 <= t <= min(T - 1, NMO + 1):
                    oproj_one(c - 1, t - 2)
                if c + 1 < NQC and T >= 3 and i < NI:
                    if t == T - 3:
                        qproj_half(c + 1, 0)
                    elif t == T - 2:
                        qproj_half(c + 1, 1)
                if c + 1 < NQC and T < 3 and i < NI and t == T - 1:
                    qproj_half(c + 1, 0)
                    qproj_half(c + 1, 1)
                # leftover O-proj tiles when T is too small to thread them all
                if c > 0 and t == T - 1 and i < NI and T - 2 < NMO:
                    for mo in range(max(T - 2, 0), NMO):
                        oproj_one(c - 1, mo)
                # next scores, then the chunk-opening PVs dead last (they
                # are gated on the previous chunk's norm)
                if i + 1 < NI:
                    scores01(*divmod(i + 1, T))
                    scoresh2(*divmod(i + 1, T))
                for ch, th, h, jh in late_pv:
                    pv_one(ch, th, h)
                    if h == HPG - 1:
                        es_hist.pop(jh)
                    if th == T - 1:
                        norm_h(ch, h)
            # tail: keep the PE HAM-warm with scratch matmuls while the
            # last norms drain on DVE, then the last chunk's O-projection
            # in pairs - both WO0 matmuls (only need CTX01) ahead of the
            # WO2 matmuls (need norm_h2), evictions alternating ACT/DVE,
            # out-DMA per pair so the store overlaps the remaining compute
            for w in range(4):
                wps = ps_w.tile([P, 512], F32, tag="psw", name=f"tw{w}")
                nc.tensor.matmul(
                    wps, lhsT=WUP[:, 0:P], rhs=WUP, start=True, stop=True
                )
            lc = NQC - 1
            c0l = lc * QC
            for mo in range(0, NMO, 2):
                pos = []
                for k in range(2):
                    po = ps_w.tile(
                        [P, 512], F32, tag="psw", name=f"tl{mo + k}"
                    )
                    nc.tensor.matmul(
                        po, lhsT=WO0[:, (mo + k) * P:(mo + k + 1) * P],
                        rhs=CTX01[:, c0l:c0l + QC],
                        start=True, stop=False,
                    )
                    pos.append(po)
                for k in range(2):
                    nc.tensor.matmul(
                        pos[k], lhsT=WO2[:, (mo + k) * P:(mo + k + 1) * P],
                        rhs=CTX2[0:DH, c0l:c0l + QC],
                        start=False, stop=True,
                    )
                ot = osb_for(lc, mo // 2)
                for k in range(2):
                    dst = ot[:, k * QC:(k + 1) * QC]
                    if k == 0:
                        nc.scalar.activation(dst, pos[k], AFT.Identity, bias=0.0)
                    else:
                        nc.vector.tensor_copy(dst, pos[k])
                oproj_pair_flush(lc, mo // 2)
    nc.compile()
    return nc


def _get_prog(KP):
    if KP not in _prog_cache:
        _prog_cache[KP] = _build_nc(KP)
    return _prog_cache[KP]


def _rearrange_w(Wslice, BF):
    # [768, 192] -> [128, 6*192] (p-major kt blocks), contiguous for DMA
    return np.ascontiguousarray(
        Wslice.reshape(DM // P, P, DQ).transpose(1, 0, 2).reshape(P, -1)
    ).astype(BF)


def _chunk_major(xT, KCH):
    # [768, KP] -> [128, NKT*KP] with per-partition layout [kc][kt][cols]
    x3 = xT.reshape(NKT, P, xT.shape[1])
    return np.concatenate(
        [
            np.ascontiguousarray(
                x3[:, :, c0:c0 + cw].transpose(1, 0, 2)
            ).reshape(P, NKT * cw)
            for c0, cw in KCH
        ],
        axis=1,
    )


def _run(inputs, trace=False):
    import ml_dtypes
    from concourse.bass_utils import run_bass_kernel_spmd

    BF = ml_dtypes.bfloat16

    query = np.asarray(inputs["query"], dtype=np.float32)
    key = np.asarray(inputs["key"], dtype=np.float32)
    value = np.asarray(inputs["value"], dtype=np.float32)
    mask = np.asarray(inputs["mask"])
    Wq = np.asarray(inputs["Wq"], dtype=np.float32)
    bq = np.asarray(inputs["bq"], dtype=np.float32)
    Wk = np.asarray(inputs["Wk"], dtype=np.float32)
    bk = np.asarray(inputs["bk"], dtype=np.float32)
    Wv = np.asarray(inputs["Wv"], dtype=np.float32)
    bv = np.asarray(inputs["bv"], dtype=np.float32)
    Wo = np.asarray(inputs["Wo"], dtype=np.float32)
    bo = np.asarray(inputs["bo"], dtype=np.float32)

    idx = [np.nonzero(mask[b, 0, 0] != 0)[0] for b in range(B)]
    keff = [len(i) for i in idx]
    KP = max(P, ((max(keff) + P - 1) // P) * P)
    T = KP // P
    KCH = [(o, min(512, KP - o)) for o in range(0, KP, 512)]

    nc = _get_prog(KP)

    per_batch = {}
    for b in range(B):
        # q: [128, NQC*NKT*QC] chunk-major ([c][kt][q])
        xqT = query[b].T.astype(BF)  # [768, 2048]
        xq_p = np.ascontiguousarray(
            xqT.reshape(NKT, P, NQC, QC).transpose(1, 2, 0, 3)
        ).reshape(P, -1)
        xkT = np.zeros((DM, KP), dtype=BF)
        xkT[:, :keff[b]] = key[b][idx[b]].T.astype(BF)
        xvT = np.zeros((DM, KP), dtype=BF)
        xvT[:, :keff[b]] = value[b][idx[b]].T.astype(BF)
        xk_p = _chunk_major(xkT, KCH)
        xv_p = _chunk_major(xvT, KCH)
        vmf = np.zeros((KP,), dtype=np.float32)
        vmf[:keff[b]] = 1.0
        vm2 = np.ascontiguousarray(vmf.reshape(T, P).T)  # [128, T]
        per_batch[b] = (xq_p, xk_p, xv_p, vm2)

    in_maps = []
    for core in range(NCORES):
        b, g = core // G, core % G
        xq_p, xk_p, xv_p, vm2 = per_batch[b]
        sl = slice(g * DQ, (g + 1) * DQ)
        wo_g = Wo[sl, :].astype(BF)  # [192, 768]
        wo_pack = np.concatenate(
            [wo_g[0:P], np.concatenate([wo_g[P:DQ], wo_g[P:DQ]], axis=0)],
            axis=1,
        )  # [128, 1536]
        mw = 4 + T + DQ
        msc = np.zeros((P, mw), dtype=np.float32)
        msc[:, 0] = bq[sl][0:P]
        msc[0:DH, 1] = bq[sl][P:DQ]
        msc[:, 2] = bk[sl][0:P]
        msc[0:DH, 3] = bk[sl][P:DQ]
        msc[:, 4:4 + T] = vm2
        msc[:, 4 + T:] = bv[sl][None, :]
        in_maps.append({
            "xq": xq_p,
            "xk": xk_p,
            "xv": xv_p,
            "wq": _rearrange_w(Wq[:, sl], BF),
            "wk": _rearrange_w(Wk[:, sl], BF),
            "wv": _rearrange_w(Wv[:, sl], BF),
            "wop": np.ascontiguousarray(wo_pack),
            "msc": msc,
        })

    res = run_bass_kernel_spmd(nc, in_maps, list(range(NCORES)), trace=trace)

    outp = np.zeros((B, S, DM), dtype=np.float32)
    for core in range(NCORES):
        o = np.asarray(res.results[core]["out"], dtype=np.float32)
        outT = o.reshape(P, NQC, NMO, QC).transpose(2, 0, 1, 3).reshape(DM, S)
        outp[core // G] += outT.T
    outp += bo.reshape(1, 1, DM)
    return outp, res


def kernel(**inputs) -> np.ndarray:
    out, _ = _run(inputs, trace=False)
    return out


if __name__ == "__main__":
    nc = _build_nc(1152)
    print("build OK")
